# revision 33
# baseline (speedup 1.0000x reference)
"""Trainium2 Bass kernel for nn_Attention (conv-qkv spatial attention block).

Contract: kernel(**inputs) takes FULL unsharded inputs (B=16, C=512, H=W=64),
shards batch across 8 NeuronCores (2 images per core), runs one SPMD Bass
program, and returns the FULL output (fp32).

Math per image (reference):
  q  = conv3x3(x, q_w) + q_b                      # (C, H, W)
  kv = conv3x3(x, kv_w) + kv_b ; k, v = split(kv)
  per channel ch: attn = softmax(q_ch @ k_ch^T) ; o_ch = attn @ v_ch
  y  = conv1x1(perm(o), proj_w) + proj_b          # head/channel permutation
       (the permutation is folded into proj_w on the host)

Device implementation notes:
  - All three 3x3 convs use 1D Winograd F(2,3) along W (1.5x fewer MACs):
    weights are G-transformed on the host (U1[j,dy] = sum_dx G1[j,dx] w),
    the input B1^T transform runs once per image on DVE (4 tensor_tensor
    ops per ci-chunk over a zero-padded copy of x) and is shared by q/k/v;
    the GEMM contracts (ci, dy) per j-position in fp16 with fp32 PSUM;
    output pairs recombine from PSUM in fp32 (y0=M0+M1+M2+b, y1=M1-M2-M3+b)
    with the bias folded into an ACT evacuation of M1.
  - Conv internals (x, weights, V) are fp16 — same PE speed as bf16 but 8x
    finer mantissa, which suppresses the Winograd noise amplification that
    would otherwise break the peaked-softmax logits. Attention operands and
    all attention matmuls stay bf16 (exp values overflow fp16 range).
  - Per-channel attention operands are produced by DVE stream-transpose
    (32x32 blocks), giving a tiled layout where the spatial index lives on
    partitions mod 32 and attention runs as K=32 matmuls packed 4-wide on
    the PE array via tile_position quadrants.
  - softmax: exp in fp32 without max subtraction (logits bounded ~|75| < 88),
    row sums via a ones-matmul, one reciprocal + broadcast multiply.
  - Keep the bias/pw DMAs on the sync (HWDGE) queue: the SWDGE (gpsimd)
    queue corrupts the rearranged bias load on the execution backend (NaN).
"""

import numpy as np
import ml_dtypes

import concourse.bass as bass
import concourse.bacc as bacc
import concourse.mybir as mybir
import concourse.tile as tile
from concourse.bass_utils import run_bass_kernel_spmd

F32 = mybir.dt.float32
BF16 = mybir.dt.bfloat16
F16 = mybir.dt.float16
AF = mybir.ActivationFunctionType
BF = ml_dtypes.bfloat16

H = 64          # spatial height (attention over rows, contracting cols)
PW = 66         # padded row width
NPOS = H * H    # 4096 positions per image


def build_nc(B=2, C=512, n_cores=8, repeat=1, phases=("conv", "attn", "proj")):
    """Build the per-core Bass program. B = images per core.

    repeat > 1 emits the whole body multiple times (timing builds only).
    phases: drop "attn"/"proj" for timing-breakdown builds.
    """
    M = C // 128            # channel chunks (co chunks and ci chunks)
    nc = bacc.Bacc("TRN2", target_bir_lowering=False, debug=False,
                   num_devices=n_cores)

    x_d = nc.dram_tensor("x", [B, C, H, H], F16, kind="ExternalInput")
    wq_d = nc.dram_tensor("wq", [M, 128, 12 * M, 128], F16, kind="ExternalInput")
    wk_d = nc.dram_tensor("wk", [M, 128, 12 * M, 128], F16, kind="ExternalInput")
    # v conv uses 1D Winograd F(2,3) along W: 12 = 4 j-positions x 3 dy taps
    wv_d = nc.dram_tensor("wv", [M, 128, 12 * M, 128], F16, kind="ExternalInput")
    pw_d = nc.dram_tensor("pw", [M, 128, C], BF16, kind="ExternalInput")
    bias_d = nc.dram_tensor("biases", [4, C], F32, kind="ExternalInput")
    y_d = nc.dram_tensor("y", [B, C, H, H], F32, kind="ExternalOutput")

    with tile.TileContext(nc) as tc:
        _body(tc, nc, B, M, x_d, (wq_d, wk_d, wv_d), pw_d, bias_d, y_d,
              repeat=repeat, phases=phases)
    nc.compile()
    return nc


def _body(tc, nc, B, M, x_d, w_ds, pw_d, bias_d, y_d, repeat=1,
          phases=("conv", "attn", "proj")):
    from contextlib import ExitStack
    ctx = ExitStack()
    C = M * 128
    const = ctx.enter_context(tc.tile_pool(name="const", bufs=1))
    xpad_p = ctx.enter_context(tc.tile_pool(name="xpad", bufs=2))
    w_p = ctx.enter_context(tc.tile_pool(name="wconv", bufs=2))
    v_p = ctx.enter_context(tc.tile_pool(name="vwino", bufs=4))
    qkv_p = ctx.enter_context(tc.tile_pool(name="qkv", bufs=4))
    exp_p = ctx.enter_context(tc.tile_pool(name="exp", bufs=3))
    nt_p = ctx.enter_context(tc.tile_pool(name="normT", bufs=1))
    rc_p = ctx.enter_context(tc.tile_pool(name="recip", bufs=1))
    as_p = ctx.enter_context(tc.tile_pool(name="attns", bufs=1))
    acm_p = ctx.enter_context(tc.tile_pool(name="attncm", bufs=min(M, 4)))
    y_p = ctx.enter_context(tc.tile_pool(name="yout", bufs=2))
    st_p = ctx.enter_context(tc.tile_pool(name="stage", bufs=4))
    cp_ps = ctx.enter_context(tc.tile_pool(name="cpps", bufs=4, space="PSUM"))
    at_ps = ctx.enter_context(tc.tile_pool(name="atps", bufs=2, space="PSUM"))
    nm_ps = ctx.enter_context(tc.tile_pool(name="nmps", bufs=1, space="PSUM"))
    o2_ps = ctx.enter_context(tc.tile_pool(name="o2ps", bufs=1, space="PSUM"))

    # constants
    ones32 = const.tile([128, 32], BF16, tag="ones32")
    nc.gpsimd.memset(ones32[:, :], 1.0)
    # per-partition bias columns: col (ti*M + m) = bias[ti, m*128:(m+1)*128]
    bias_sb = const.tile([128, 4 * M], F32, tag="bias")
    nc.sync.dma_start(out=bias_sb[:, :],
                      in_=bias_d[:, :].rearrange("a (m p) -> p (a m)", p=128))
    pw_sb = const.tile([128, M * C], BF16, tag="pw")
    for k4 in range(M):
        nc.sync.dma_start(out=pw_sb[:, k4 * C:(k4 + 1) * C], in_=pw_d[k4, :, :])

    for b in [b for _ in range(repeat) for b in range(B)]:
        # ---- load x image b (fp16): zero-pad borders, then 1D Winograd
        # B1^T input transform along W (4 DVE ops per ci-chunk), shared by
        # the q/k/v GEMMs. V[j, r', tw] in fp16.
        SUB = mybir.AluOpType.subtract
        ADD = mybir.AluOpType.add
        vts = []
        for k4 in range(M):
            xp = xpad_p.tile([128, PW * PW], F16, tag="xpad")
            z = xp[:, :].rearrange("p (r c) -> p r c", c=PW)
            nc.gpsimd.memset(z[:, 0, :], 0.0)
            nc.gpsimd.memset(z[:, PW - 1, :], 0.0)
            nc.gpsimd.memset(z[:, :, 0], 0.0)
            nc.gpsimd.memset(z[:, :, PW - 1], 0.0)
            nc.sync.dma_start(out=z[:, 1:H + 1, 1:H + 1],
                              in_=x_d[b, k4 * 128:(k4 + 1) * 128, :, :])
            vt = v_p.tile([128, 4 * PW * 32], F16, tag="vtile")
            vz = vt[:, :].rearrange("p (j r c) -> p j r c", j=4, r=PW)
            z2 = xp[:, :].rearrange("p (r c2 par) -> p r c2 par", par=2, c2=33)
            dA0 = z2[:, :, 0:32, 0]       # cp = 2tw
            dA1 = z2[:, :, 0:32, 1]       # cp = 2tw+1
            dB0 = z2[:, :, 1:33, 0]       # cp = 2tw+2
            dB1 = z2[:, :, 1:33, 1]       # cp = 2tw+3
            nc.vector.tensor_tensor(vz[:, 0], dA0, dB0, SUB)
            nc.vector.tensor_tensor(vz[:, 1], dA1, dB0, ADD)
            nc.vector.tensor_tensor(vz[:, 2], dB0, dA1, SUB)
            nc.vector.tensor_tensor(vz[:, 3], dA1, dB1, SUB)
            vts.append(vz)
            if k4 == 0:
                w_pre = w_p.tile([128, 12 * M * 128], F16, tag="wconv")
                nc.sync.dma_start(
                    out=w_pre[:, :],
                    in_=w_ds[0][0].rearrange("p a b -> p (a b)"))

        acm = []  # attnout channel-major chunks for proj
        for m in range(M):
            outs = {}

            def vgemm(wt, j, rsl, csl, cpr):
                """Accumulate M_j = sum_{dy,ci} U1[j,dy] @ V[j] into a psum.
                rsl/csl slice V rows/tile-cols; cpr=True puts tw outer
                (col-major free order for the v staging)."""
                ps = cp_ps.tile([128, 512], F32, tag="cpps")
                for i2, (k4, dy) in enumerate(
                        (k, d) for k in range(M) for d in range(3)):
                    rhs = vts[k4][:, j, dy + rsl:dy + rsl + (64 if cpr else 16),
                                  csl:csl + (8 if cpr else 32)]
                    if cpr:
                        rhs = rhs.transpose([0, 2, 1])
                    nc.tensor.matmul(
                        ps[:, :],
                        wt[:, ((j * 3 + dy) * M + k4) * 128:
                           ((j * 3 + dy) * M + k4 + 1) * 128],
                        rhs, start=(i2 == 0), stop=(i2 == 3 * M - 1))
                return ps

            def ycombine(wt, tb, o_t, qt, cpr):
                """Winograd output recombine for one psum group -> 2 STs.
                y0 = M0+M1+M2+b ; y1 = M1-M2-M3+b (fp32 PSUM reads)."""
                rsl = 0 if cpr else qt * 16
                csl = qt * 8 if cpr else 0
                shp = (lambda t: t.rearrange("p (tw r) -> p tw r", tw=8)) \
                    if cpr else \
                    (lambda t: t.rearrange("p (r tw) -> p r tw", tw=32))
                m1 = vgemm(wt, 1, rsl, csl, cpr)
                ev1 = st_p.tile([128, 512], BF16, tag="stage")
                nc.scalar.activation(ev1[:, :], m1[:, :], AF.Identity, bias=tb)
                m2 = vgemm(wt, 2, rsl, csl, cpr)
                tws = st_p.tile([128, 512], BF16, tag="stage")
                twd = st_p.tile([128, 512], BF16, tag="stage")
                nc.vector.tensor_tensor(shp(tws[:, :]), shp(ev1[:, :]),
                                        shp(m2[:, :]), ADD)
                nc.vector.tensor_tensor(shp(twd[:, :]), shp(ev1[:, :]),
                                        shp(m2[:, :]), SUB)
                m0 = vgemm(wt, 0, rsl, csl, cpr)
                m3 = vgemm(wt, 3, rsl, csl, cpr)
                stg = st_p.tile([128, 1024], BF16, tag="stage2")
                if cpr:   # col-major: free = (c 16, r 64), c = 2tw+q
                    sz = stg[:, :].rearrange("p (tw q r) -> p tw q r",
                                             tw=8, q=2)
                    y0, y1 = sz[:, :, 0, :], sz[:, :, 1, :]
                else:     # row-major: free = (r 16, c 64), c = 2c2+q
                    sz = stg[:, :].rearrange("p (r c2 q) -> p r c2 q",
                                             q=2, c2=32)
                    y0, y1 = sz[:, :, :, 0], sz[:, :, :, 1]
                nc.vector.tensor_tensor(y0, shp(tws[:, :]), shp(m0[:, :]), ADD)
                nc.vector.tensor_tensor(y1, shp(twd[:, :]), shp(m3[:, :]), SUB)
                nc.vector.transpose(o_t[:, (2 * qt) * 512:(2 * qt + 1) * 512],
                                    stg[:, 0:512])
                nc.vector.transpose(o_t[:, (2 * qt + 1) * 512:
                                        (2 * qt + 2) * 512],
                                    stg[:, 512:1024])

            for ti, tname in enumerate(("q", "k", "v")):
                if m == 0 and ti == 0:
                    wt = w_pre
                else:
                    wt = w_p.tile([128, 12 * M * 128], F16, tag="wconv")
                    nc.sync.dma_start(
                        out=wt[:, :],
                        in_=w_ds[ti][m].rearrange("p a b -> p (a b)"))
                o_t = qkv_p.tile([128, NPOS], BF16, tag="qkv")
                tb = bias_sb[:, ti * M + m: ti * M + m + 1]
                for qt in range(4):
                    ycombine(wt, tb, o_t, qt, cpr=(tname == "v"))
                outs[tname] = o_t

            if "attn" not in phases:
                nc.gpsimd.dma_start(
                    out=y_d[b, m * 128:(m + 1) * 128].rearrange("p a b -> p (a b)"),
                    in_=outs["q"][:, :])
                continue

            # ---- attention for the 128 channels of chunk m
            o_q, o_k, o_v = outs["q"], outs["k"], outs["v"]
            a_s = as_p.tile([128, NPOS], BF16, tag="attns")
            kks = [o_k[cb * 32:(cb + 1) * 32, :].rearrange(
                "p (kid half c) -> p kid half c", half=2, c=32) for cb in range(4)]
            qqs = [o_q[cb * 32:(cb + 1) * 32, :].rearrange(
                "p (i half c) -> p i half c", half=2, c=32) for cb in range(4)]
            vvs = [o_v[cb * 32:(cb + 1) * 32, :].rearrange(
                "p (w half c) -> p w half c", half=2, c=32) for cb in range(4)]
            for qd in range(8):
                atp = at_ps.tile([128, 512], F32, tag="atps")
                # logits^T:  atp[cb*32+kappa, sl*128+kb*64+i] = sum_j k*q
                # cb innermost so consecutive MMs hit different PE quadrants
                for i1, (sl, kb, jb) in enumerate(
                        (s, k, j) for s in range(4) for k in range(2) for j in range(2)):
                    c = qd * 4 + sl
                    for cb in range(4):
                        nc.tensor.matmul(
                            atp[cb * 32:(cb + 1) * 32,
                                sl * 128 + kb * 64: sl * 128 + (kb + 1) * 64],
                            kks[cb][:, kb * 32:(kb + 1) * 32, jb, c],
                            qqs[cb][:, :, jb, c],
                            start=(i1 == 0), stop=(i1 == 15),
                            skip_group_check=True,
                            tile_position=(cb * 32, cb * 32))
                # exp (fp32 -> bf16), no max subtraction
                ex = exp_p.tile([128, 512], BF16, tag="exp")
                nc.scalar.activation(ex[:, :], atp[:, :], AF.Exp)
                # row sums (over kidx) via ones-matmul, replicated on 32 parts
                nmp = nm_ps.tile([128, 256], F32, tag="nmps")
                for kb in range(2):
                    for cb in range(4):
                        ee = ex[cb * 32:(cb + 1) * 32, :].rearrange(
                            "p (sl half i) -> p sl half i", half=2, i=64)
                        nc.tensor.matmul(
                            nmp[cb * 32:(cb + 1) * 32, :],
                            ones32[cb * 32:(cb + 1) * 32, :],
                            ee[:, :, kb, :],
                            start=(kb == 0), stop=(kb == 1),
                            skip_group_check=True,
                            tile_position=(cb * 32, cb * 32))
                nt = nt_p.tile([128, 256], F32, tag="normT")
                nc.vector.transpose(nt[:, :], nmp[:, :])
                rc = rc_p.tile([128, 8], F32, tag="recip")
                nc.vector.reciprocal(
                    rc[:, :], nt[:, :].rearrange("p (t c) -> p t c", c=32)[:, :, 0])
                # out2 = attn_exp^T' @ v   (unnormalized), K=32 chunks
                o2p = o2_ps.tile([128, 512], F32, tag="o2ps")
                for i2, (sl, ib, kb) in enumerate(
                        (s, i, k) for s in range(4) for i in range(2) for k in range(2)):
                    c = qd * 4 + sl
                    for cb in range(4):
                        nc.tensor.matmul(
                            o2p[cb * 32:(cb + 1) * 32,
                                sl * 128 + ib * 64: sl * 128 + (ib + 1) * 64],
                            ex[cb * 32:(cb + 1) * 32,
                               sl * 128 + kb * 64 + ib * 32:
                               sl * 128 + kb * 64 + ib * 32 + 32],
                            vvs[cb][:, :, kb, c],
                            start=(i2 == 0), stop=(i2 == 15),
                            skip_group_check=True,
                            tile_position=(cb * 32, cb * 32))
                # normalize + write into attnout_s (v-style layout), bf16
                in0 = o2p[:, :].rearrange("p (sl ib w) -> p sl ib w", ib=2, w=64)
                in1 = rc[:, :].rearrange("p (sl ib) -> p sl ib", ib=2)
                in1 = in1.unsqueeze(3).broadcast_to((128, 4, 2, 64))
                outap = a_s[:, :].rearrange("p (t c) -> p t c", c=32)
                outap = outap[:, :, qd * 4:qd * 4 + 4].rearrange(
                    "p (w ib) sl -> p w ib sl", ib=2).transpose([0, 3, 2, 1])
                nc.vector.tensor_tensor(outap, in0, in1, mybir.AluOpType.mult)
            # back-transpose to channel-major (column-major positions)
            a_cm = acm_p.tile([128, NPOS], BF16, tag="attncm")
            nc.vector.transpose(a_cm[:, :], a_s[:, :])
            acm.append(a_cm)

        if "attn" not in phases:
            continue
        if "proj" not in phases:
            for m in range(M):
                nc.gpsimd.dma_start(
                    out=y_d[b, m * 128:(m + 1) * 128].rearrange("p a b -> p (a b)"),
                    in_=acm[m][:, :])
            del acm
            continue

        # ---- proj (1x1 conv with permuted weights) + bias, row-major out.
        # y-writes batched in pairs of psum groups (one 512 KB DMA per 16
        # output rows) to halve the per-DMA overhead on the DMA engines.
        for mo in range(M):
            for n2 in range(4):
                yt = y_p.tile([128, NPOS // 4], F32, tag="yout")
                for half in range(2):
                    n = n2 * 2 + half
                    psum = cp_ps.tile([128, 512], F32, tag="cpps")
                    for k4 in range(M):
                        rhs = acm[k4][:, :].rearrange("p (w i) -> p w i", i=64)
                        rhs = rhs[:, :, n * 8:(n + 1) * 8].transpose([0, 2, 1])
                        nc.tensor.matmul(
                            psum[:, :],
                            pw_sb[:, k4 * C + mo * 128: k4 * C + (mo + 1) * 128],
                            rhs, start=(k4 == 0), stop=(k4 == M - 1))
                    nc.scalar.activation(
                        yt[:, half * 512:(half + 1) * 512], psum[:, :],
                        AF.Identity,
                        bias=bias_sb[:, 3 * M + mo: 3 * M + mo + 1])
                nc.sync.dma_start(
                    out=y_d[b, mo * 128:(mo + 1) * 128,
                            n2 * 16:(n2 + 1) * 16, :],
                    in_=yt[:, :])
        del acm
    ctx.close()


def prep_weights(q_w, q_b, kv_w, kv_b, proj_w, proj_b, C=512):
    """Host-side weight re-layouts (numpy, bf16)."""
    M = C // 128
    nh = 16
    cpg = C // nh

    def conv_w(w):
        # w[co, ci, dy, dx] -> [m, p(ci%128), t(=dy*3+dx), k4, co] flat
        w4 = w.reshape(M, 128, M, 128, 3, 3)          # [m, co, k4, p, dy, dx]
        out = np.transpose(w4, (0, 3, 4, 5, 2, 1))    # [m, p, dy, dx, k4, co]
        out = out.reshape(M, 128, 9 * M, 128)
        return np.ascontiguousarray(out).astype(BF)

    def conv_w_wino(w):
        # 1D Winograd F(2,3) along W: U1[j,dy,o,c] = sum_dx G1[j,dx] w[o,c,dy,dx]
        # layout [m, p(ci%128), t'(=j*3+dy), k4, co] flat
        G1 = np.array([[1, 0, 0], [.5, .5, .5], [.5, -.5, .5], [0, 0, 1]],
                      np.float32)
        U1 = np.einsum('jx,ocdx->ocjd', G1, w.astype(np.float32))
        w4 = U1.reshape(M, 128, M, 128, 4, 3)         # [m, co, k4, p, j, dy]
        out = np.transpose(w4, (0, 3, 4, 5, 2, 1))    # [m, p, j, dy, k4, co]
        out = out.reshape(M, 128, 12 * M, 128)
        return np.ascontiguousarray(out).astype(np.float16)

    wq = conv_w_wino(q_w)
    wk = conv_w_wino(kv_w[:C])
    wv = conv_w_wino(kv_w[C:])
    ch = np.arange(C)
    perm = (ch % cpg) * nh + ch // cpg                # proj input index per attn channel
    pwp = proj_w[:, :, 0, 0][:, perm]                 # [co, ch]
    pw = np.ascontiguousarray(pwp.T.reshape(M, 128, C)).astype(BF)
    biases = np.stack([q_b, kv_b[:C], kv_b[C:], proj_b]).astype(np.float32)
    return wq, wk, wv, pw, biases


_CACHE = {}


def _get_nc():
    if "nc" not in _CACHE:
        _CACHE["nc"] = build_nc(B=2, C=512, n_cores=8)
    return _CACHE["nc"]


def make_in_maps(x, q_w, q_b, kv_w, kv_b, proj_w, proj_b, n_cores=8):
    wq, wk, wv, pw, biases = prep_weights(
        np.asarray(q_w), np.asarray(q_b), np.asarray(kv_w), np.asarray(kv_b),
        np.asarray(proj_w), np.asarray(proj_b))
    x = np.asarray(x, dtype=np.float32).astype(np.float16)
    bpc = x.shape[0] // n_cores
    return [
        {"x": np.ascontiguousarray(x[i * bpc:(i + 1) * bpc]),
         "wq": wq, "wk": wk, "wv": wv, "pw": pw, "biases": biases}
        for i in range(n_cores)
    ]


def kernel(x, q_w, q_b, kv_w, kv_b, proj_w, proj_b):
    nc = _get_nc()
    in_maps = make_in_maps(x, q_w, q_b, kv_w, kv_b, proj_w, proj_b)
    res = run_bass_kernel_spmd(nc, in_maps, core_ids=list(range(8)))
    out = np.concatenate([res.results[i]["y"] for i in range(8)], axis=0)
    return out.astype(np.float32)



# revision 34
# speedup vs baseline: 1.0065x; 1.0065x over previous
"""Trainium2 Bass kernel for nn_Attention (conv-qkv spatial attention block).

Contract: kernel(**inputs) takes FULL unsharded inputs (B=16, C=512, H=W=64),
shards batch across 8 NeuronCores (2 images per core), runs one SPMD Bass
program, and returns the FULL output (fp32).

Math per image (reference):
  q  = conv3x3(x, q_w) + q_b                      # (C, H, W)
  kv = conv3x3(x, kv_w) + kv_b ; k, v = split(kv)
  per channel ch: attn = softmax(q_ch @ k_ch^T) ; o_ch = attn @ v_ch
  y  = conv1x1(perm(o), proj_w) + proj_b          # head/channel permutation
       (the permutation is folded into proj_w on the host)

Device implementation notes:
  - All three 3x3 convs use 1D Winograd F(2,3) along W (1.5x fewer MACs):
    weights are G-transformed on the host (U1[j,dy] = sum_dx G1[j,dx] w),
    the input B1^T transform runs once per image on DVE (4 tensor_tensor
    ops per ci-chunk over a zero-padded copy of x) and is shared by q/k/v;
    the GEMM contracts (ci, dy) per j-position in fp16 with fp32 PSUM;
    output pairs recombine from PSUM in fp32 (y0=M0+M1+M2+b, y1=M1-M2-M3+b)
    with the bias folded into an ACT evacuation of M1.
  - Conv internals (x, weights, V) are fp16 — same PE speed as bf16 but 8x
    finer mantissa, which suppresses the Winograd noise amplification that
    would otherwise break the peaked-softmax logits. Attention operands and
    all attention matmuls stay bf16 (exp values overflow fp16 range).
  - Per-channel attention operands are produced by DVE stream-transpose
    (32x32 blocks), giving a tiled layout where the spatial index lives on
    partitions mod 32 and attention runs as K=32 matmuls packed 4-wide on
    the PE array via tile_position quadrants.
  - softmax: exp in fp32 without max subtraction (logits bounded ~|75| < 88),
    row sums via a ones-matmul, one reciprocal + broadcast multiply.
  - Keep the bias/pw DMAs on the sync (HWDGE) queue: the SWDGE (gpsimd)
    queue corrupts the rearranged bias load on the execution backend (NaN).
"""

import numpy as np
import ml_dtypes

import concourse.bass as bass
import concourse.bacc as bacc
import concourse.mybir as mybir
import concourse.tile as tile
from concourse.bass_utils import run_bass_kernel_spmd

F32 = mybir.dt.float32
BF16 = mybir.dt.bfloat16
F16 = mybir.dt.float16
AF = mybir.ActivationFunctionType
BF = ml_dtypes.bfloat16

H = 64          # spatial height (attention over rows, contracting cols)
PW = 66         # padded row width
NPOS = H * H    # 4096 positions per image


def build_nc(B=2, C=512, n_cores=8, repeat=1, phases=("conv", "attn", "proj")):
    """Build the per-core Bass program. B = images per core.

    repeat > 1 emits the whole body multiple times (timing builds only).
    phases: drop "attn"/"proj" for timing-breakdown builds.
    """
    M = C // 128            # channel chunks (co chunks and ci chunks)
    nc = bacc.Bacc("TRN2", target_bir_lowering=False, debug=False,
                   num_devices=n_cores)

    x_d = nc.dram_tensor("x", [B, C, H, H], F16, kind="ExternalInput")
    wq_d = nc.dram_tensor("wq", [M, 128, 12 * M, 128], F16, kind="ExternalInput")
    wk_d = nc.dram_tensor("wk", [M, 128, 12 * M, 128], F16, kind="ExternalInput")
    # v conv uses 1D Winograd F(2,3) along W: 12 = 4 j-positions x 3 dy taps
    wv_d = nc.dram_tensor("wv", [M, 128, 12 * M, 128], F16, kind="ExternalInput")
    pw_d = nc.dram_tensor("pw", [M, 128, C], BF16, kind="ExternalInput")
    bias_d = nc.dram_tensor("biases", [4, C], F32, kind="ExternalInput")
    y_d = nc.dram_tensor("y", [B, C, H, H], F32, kind="ExternalOutput")

    with tile.TileContext(nc) as tc:
        _body(tc, nc, B, M, x_d, (wq_d, wk_d, wv_d), pw_d, bias_d, y_d,
              repeat=repeat, phases=phases)
    nc.compile()
    return nc


def _body(tc, nc, B, M, x_d, w_ds, pw_d, bias_d, y_d, repeat=1,
          phases=("conv", "attn", "proj")):
    from contextlib import ExitStack
    ctx = ExitStack()
    C = M * 128
    const = ctx.enter_context(tc.tile_pool(name="const", bufs=1))
    xpad_p = ctx.enter_context(tc.tile_pool(name="xpad", bufs=2))
    w_p = ctx.enter_context(tc.tile_pool(name="wconv", bufs=2))
    v_p = ctx.enter_context(tc.tile_pool(name="vwino", bufs=4))
    qkv_p = ctx.enter_context(tc.tile_pool(name="qkv", bufs=4))
    exp_p = ctx.enter_context(tc.tile_pool(name="exp", bufs=3))
    nt_p = ctx.enter_context(tc.tile_pool(name="normT", bufs=1))
    rc_p = ctx.enter_context(tc.tile_pool(name="recip", bufs=1))
    as_p = ctx.enter_context(tc.tile_pool(name="attns", bufs=1))
    acm_p = ctx.enter_context(tc.tile_pool(name="attncm", bufs=min(M, 4)))
    y_p = ctx.enter_context(tc.tile_pool(name="yout", bufs=2))
    st_p = ctx.enter_context(tc.tile_pool(name="stage", bufs=4))
    cp_ps = ctx.enter_context(tc.tile_pool(name="cpps", bufs=4, space="PSUM"))
    at_ps = ctx.enter_context(tc.tile_pool(name="atps", bufs=2, space="PSUM"))
    nm_ps = ctx.enter_context(tc.tile_pool(name="nmps", bufs=1, space="PSUM"))
    o2_ps = ctx.enter_context(tc.tile_pool(name="o2ps", bufs=1, space="PSUM"))

    # constants
    ones32 = const.tile([128, 32], BF16, tag="ones32")
    nc.gpsimd.memset(ones32[:, :], 1.0)
    # per-partition bias columns: col (ti*M + m) = bias[ti, m*128:(m+1)*128]
    bias_sb = const.tile([128, 4 * M], F32, tag="bias")
    nc.sync.dma_start(out=bias_sb[:, :],
                      in_=bias_d[:, :].rearrange("a (m p) -> p (a m)", p=128))
    pw_sb = const.tile([128, M * C], BF16, tag="pw")
    for k4 in range(M):
        nc.sync.dma_start(out=pw_sb[:, k4 * C:(k4 + 1) * C], in_=pw_d[k4, :, :])

    for b in [b for _ in range(repeat) for b in range(B)]:
        # ---- load x image b (fp16): zero-pad borders, then 1D Winograd
        # B1^T input transform along W (4 DVE ops per ci-chunk), shared by
        # the q/k/v GEMMs. V[j, r', tw] in fp16.
        SUB = mybir.AluOpType.subtract
        ADD = mybir.AluOpType.add
        vts = []
        for k4 in range(M):
            xp = xpad_p.tile([128, PW * PW], F16, tag="xpad")
            z = xp[:, :].rearrange("p (r c) -> p r c", c=PW)
            nc.gpsimd.memset(z[:, 0, :], 0.0)
            nc.gpsimd.memset(z[:, PW - 1, :], 0.0)
            nc.gpsimd.memset(z[:, :, 0], 0.0)
            nc.gpsimd.memset(z[:, :, PW - 1], 0.0)
            # x rows split in halves so transforms (and the first GEMM
            # groups, which read only low V rows) start before the full
            # image lands
            nc.sync.dma_start(out=z[:, 1:33, 1:H + 1],
                              in_=x_d[b, k4 * 128:(k4 + 1) * 128, 0:32, :])
            nc.sync.dma_start(out=z[:, 33:H + 1, 1:H + 1],
                              in_=x_d[b, k4 * 128:(k4 + 1) * 128, 32:H, :])
            vt = v_p.tile([128, 4 * PW * 32], F16, tag="vtile")
            vz = vt[:, :].rearrange("p (j r c) -> p j r c", j=4, r=PW)
            z2 = xp[:, :].rearrange("p (r c2 par) -> p r c2 par", par=2, c2=33)
            for r0, r1 in ((0, 33), (33, PW)):
                dA0 = z2[:, r0:r1, 0:32, 0]       # cp = 2tw
                dA1 = z2[:, r0:r1, 0:32, 1]       # cp = 2tw+1
                dB0 = z2[:, r0:r1, 1:33, 0]       # cp = 2tw+2
                dB1 = z2[:, r0:r1, 1:33, 1]       # cp = 2tw+3
                nc.vector.tensor_tensor(vz[:, 0, r0:r1], dA0, dB0, SUB)
                nc.vector.tensor_tensor(vz[:, 1, r0:r1], dA1, dB0, ADD)
                nc.vector.tensor_tensor(vz[:, 2, r0:r1], dB0, dA1, SUB)
                nc.vector.tensor_tensor(vz[:, 3, r0:r1], dA1, dB1, SUB)
            vts.append(vz)
            if k4 == 0:
                w_pre = w_p.tile([128, 12 * M * 128], F16, tag="wconv")
                nc.sync.dma_start(
                    out=w_pre[:, :],
                    in_=w_ds[0][0].rearrange("p a b -> p (a b)"))

        acm = []  # attnout channel-major chunks for proj
        for m in range(M):
            outs = {}

            def vgemm(wt, j, rsl, csl, cpr):
                """Accumulate M_j = sum_{dy,ci} U1[j,dy] @ V[j] into a psum.
                rsl/csl slice V rows/tile-cols; cpr=True puts tw outer
                (col-major free order for the v staging)."""
                ps = cp_ps.tile([128, 512], F32, tag="cpps")
                for i2, (k4, dy) in enumerate(
                        (k, d) for k in range(M) for d in range(3)):
                    rhs = vts[k4][:, j, dy + rsl:dy + rsl + (64 if cpr else 16),
                                  csl:csl + (8 if cpr else 32)]
                    if cpr:
                        rhs = rhs.transpose([0, 2, 1])
                    nc.tensor.matmul(
                        ps[:, :],
                        wt[:, ((j * 3 + dy) * M + k4) * 128:
                           ((j * 3 + dy) * M + k4 + 1) * 128],
                        rhs, start=(i2 == 0), stop=(i2 == 3 * M - 1))
                return ps

            def ycombine(wt, tb, o_t, qt, cpr):
                """Winograd output recombine for one psum group -> 2 STs.
                y0 = M0+M1+M2+b ; y1 = M1-M2-M3+b (fp32 PSUM reads)."""
                rsl = 0 if cpr else qt * 16
                csl = qt * 8 if cpr else 0
                shp = (lambda t: t.rearrange("p (tw r) -> p tw r", tw=8)) \
                    if cpr else \
                    (lambda t: t.rearrange("p (r tw) -> p r tw", tw=32))
                m1 = vgemm(wt, 1, rsl, csl, cpr)
                ev1 = st_p.tile([128, 512], BF16, tag="stage")
                nc.scalar.activation(ev1[:, :], m1[:, :], AF.Identity, bias=tb)
                m2 = vgemm(wt, 2, rsl, csl, cpr)
                tws = st_p.tile([128, 512], BF16, tag="stage")
                twd = st_p.tile([128, 512], BF16, tag="stage")
                nc.vector.tensor_tensor(shp(tws[:, :]), shp(ev1[:, :]),
                                        shp(m2[:, :]), ADD)
                nc.vector.tensor_tensor(shp(twd[:, :]), shp(ev1[:, :]),
                                        shp(m2[:, :]), SUB)
                m0 = vgemm(wt, 0, rsl, csl, cpr)
                m3 = vgemm(wt, 3, rsl, csl, cpr)
                stg = st_p.tile([128, 1024], BF16, tag="stage2")
                if cpr:   # col-major: free = (c 16, r 64), c = 2tw+q
                    sz = stg[:, :].rearrange("p (tw q r) -> p tw q r",
                                             tw=8, q=2)
                    y0, y1 = sz[:, :, 0, :], sz[:, :, 1, :]
                else:     # row-major: free = (r 16, c 64), c = 2c2+q
                    sz = stg[:, :].rearrange("p (r c2 q) -> p r c2 q",
                                             q=2, c2=32)
                    y0, y1 = sz[:, :, :, 0], sz[:, :, :, 1]
                nc.vector.tensor_tensor(y0, shp(tws[:, :]), shp(m0[:, :]), ADD)
                nc.vector.tensor_tensor(y1, shp(twd[:, :]), shp(m3[:, :]), SUB)
                nc.vector.transpose(o_t[:, (2 * qt) * 512:(2 * qt + 1) * 512],
                                    stg[:, 0:512])
                nc.vector.transpose(o_t[:, (2 * qt + 1) * 512:
                                        (2 * qt + 2) * 512],
                                    stg[:, 512:1024])

            for ti, tname in enumerate(("q", "k", "v")):
                if m == 0 and ti == 0:
                    wt = w_pre
                else:
                    wt = w_p.tile([128, 12 * M * 128], F16, tag="wconv")
                    nc.sync.dma_start(
                        out=wt[:, :],
                        in_=w_ds[ti][m].rearrange("p a b -> p (a b)"))
                o_t = qkv_p.tile([128, NPOS], BF16, tag="qkv")
                tb = bias_sb[:, ti * M + m: ti * M + m + 1]
                for qt in range(4):
                    ycombine(wt, tb, o_t, qt, cpr=(tname == "v"))
                outs[tname] = o_t

            if "attn" not in phases:
                nc.gpsimd.dma_start(
                    out=y_d[b, m * 128:(m + 1) * 128].rearrange("p a b -> p (a b)"),
                    in_=outs["q"][:, :])
                continue

            # ---- attention for the 128 channels of chunk m
            o_q, o_k, o_v = outs["q"], outs["k"], outs["v"]
            a_s = as_p.tile([128, NPOS], BF16, tag="attns")
            kks = [o_k[cb * 32:(cb + 1) * 32, :].rearrange(
                "p (kid half c) -> p kid half c", half=2, c=32) for cb in range(4)]
            qqs = [o_q[cb * 32:(cb + 1) * 32, :].rearrange(
                "p (i half c) -> p i half c", half=2, c=32) for cb in range(4)]
            vvs = [o_v[cb * 32:(cb + 1) * 32, :].rearrange(
                "p (w half c) -> p w half c", half=2, c=32) for cb in range(4)]
            for qd in range(8):
                atp = at_ps.tile([128, 512], F32, tag="atps")
                # logits^T:  atp[cb*32+kappa, sl*128+kb*64+i] = sum_j k*q
                # cb innermost so consecutive MMs hit different PE quadrants
                for i1, (sl, kb, jb) in enumerate(
                        (s, k, j) for s in range(4) for k in range(2) for j in range(2)):
                    c = qd * 4 + sl
                    for cb in range(4):
                        nc.tensor.matmul(
                            atp[cb * 32:(cb + 1) * 32,
                                sl * 128 + kb * 64: sl * 128 + (kb + 1) * 64],
                            kks[cb][:, kb * 32:(kb + 1) * 32, jb, c],
                            qqs[cb][:, :, jb, c],
                            start=(i1 == 0), stop=(i1 == 15),
                            skip_group_check=True,
                            tile_position=(cb * 32, cb * 32))
                # exp (fp32 -> bf16), no max subtraction
                ex = exp_p.tile([128, 512], BF16, tag="exp")
                nc.scalar.activation(ex[:, :], atp[:, :], AF.Exp)
                # row sums (over kidx) via ones-matmul, replicated on 32 parts
                nmp = nm_ps.tile([128, 256], F32, tag="nmps")
                for kb in range(2):
                    for cb in range(4):
                        ee = ex[cb * 32:(cb + 1) * 32, :].rearrange(
                            "p (sl half i) -> p sl half i", half=2, i=64)
                        nc.tensor.matmul(
                            nmp[cb * 32:(cb + 1) * 32, :],
                            ones32[cb * 32:(cb + 1) * 32, :],
                            ee[:, :, kb, :],
                            start=(kb == 0), stop=(kb == 1),
                            skip_group_check=True,
                            tile_position=(cb * 32, cb * 32))
                nt = nt_p.tile([128, 256], F32, tag="normT")
                nc.vector.transpose(nt[:, :], nmp[:, :])
                rc = rc_p.tile([128, 8], F32, tag="recip")
                nc.vector.reciprocal(
                    rc[:, :], nt[:, :].rearrange("p (t c) -> p t c", c=32)[:, :, 0])
                # out2 = attn_exp^T' @ v   (unnormalized), K=32 chunks
                o2p = o2_ps.tile([128, 512], F32, tag="o2ps")
                for i2, (sl, ib, kb) in enumerate(
                        (s, i, k) for s in range(4) for i in range(2) for k in range(2)):
                    c = qd * 4 + sl
                    for cb in range(4):
                        nc.tensor.matmul(
                            o2p[cb * 32:(cb + 1) * 32,
                                sl * 128 + ib * 64: sl * 128 + (ib + 1) * 64],
                            ex[cb * 32:(cb + 1) * 32,
                               sl * 128 + kb * 64 + ib * 32:
                               sl * 128 + kb * 64 + ib * 32 + 32],
                            vvs[cb][:, :, kb, c],
                            start=(i2 == 0), stop=(i2 == 15),
                            skip_group_check=True,
                            tile_position=(cb * 32, cb * 32))
                # normalize + write into attnout_s (v-style layout), bf16
                in0 = o2p[:, :].rearrange("p (sl ib w) -> p sl ib w", ib=2, w=64)
                in1 = rc[:, :].rearrange("p (sl ib) -> p sl ib", ib=2)
                in1 = in1.unsqueeze(3).broadcast_to((128, 4, 2, 64))
                outap = a_s[:, :].rearrange("p (t c) -> p t c", c=32)
                outap = outap[:, :, qd * 4:qd * 4 + 4].rearrange(
                    "p (w ib) sl -> p w ib sl", ib=2).transpose([0, 3, 2, 1])
                nc.vector.tensor_tensor(outap, in0, in1, mybir.AluOpType.mult)
            # back-transpose to channel-major (column-major positions)
            a_cm = acm_p.tile([128, NPOS], BF16, tag="attncm")
            nc.vector.transpose(a_cm[:, :], a_s[:, :])
            acm.append(a_cm)

        if "attn" not in phases:
            continue
        if "proj" not in phases:
            for m in range(M):
                nc.gpsimd.dma_start(
                    out=y_d[b, m * 128:(m + 1) * 128].rearrange("p a b -> p (a b)"),
                    in_=acm[m][:, :])
            del acm
            continue

        # ---- proj (1x1 conv with permuted weights) + bias, row-major out.
        # y-writes batched in pairs of psum groups (one 512 KB DMA per 16
        # output rows) to halve the per-DMA overhead on the DMA engines.
        for mo in range(M):
            for n2 in range(4):
                yt = y_p.tile([128, NPOS // 4], F32, tag="yout")
                for half in range(2):
                    n = n2 * 2 + half
                    psum = cp_ps.tile([128, 512], F32, tag="cpps")
                    for k4 in range(M):
                        rhs = acm[k4][:, :].rearrange("p (w i) -> p w i", i=64)
                        rhs = rhs[:, :, n * 8:(n + 1) * 8].transpose([0, 2, 1])
                        nc.tensor.matmul(
                            psum[:, :],
                            pw_sb[:, k4 * C + mo * 128: k4 * C + (mo + 1) * 128],
                            rhs, start=(k4 == 0), stop=(k4 == M - 1))
                    nc.scalar.activation(
                        yt[:, half * 512:(half + 1) * 512], psum[:, :],
                        AF.Identity,
                        bias=bias_sb[:, 3 * M + mo: 3 * M + mo + 1])
                nc.sync.dma_start(
                    out=y_d[b, mo * 128:(mo + 1) * 128,
                            n2 * 16:(n2 + 1) * 16, :],
                    in_=yt[:, :])
        del acm
    ctx.close()


def prep_weights(q_w, q_b, kv_w, kv_b, proj_w, proj_b, C=512):
    """Host-side weight re-layouts (numpy, bf16)."""
    M = C // 128
    nh = 16
    cpg = C // nh

    def conv_w(w):
        # w[co, ci, dy, dx] -> [m, p(ci%128), t(=dy*3+dx), k4, co] flat
        w4 = w.reshape(M, 128, M, 128, 3, 3)          # [m, co, k4, p, dy, dx]
        out = np.transpose(w4, (0, 3, 4, 5, 2, 1))    # [m, p, dy, dx, k4, co]
        out = out.reshape(M, 128, 9 * M, 128)
        return np.ascontiguousarray(out).astype(BF)

    def conv_w_wino(w):
        # 1D Winograd F(2,3) along W: U1[j,dy,o,c] = sum_dx G1[j,dx] w[o,c,dy,dx]
        # layout [m, p(ci%128), t'(=j*3+dy), k4, co] flat
        G1 = np.array([[1, 0, 0], [.5, .5, .5], [.5, -.5, .5], [0, 0, 1]],
                      np.float32)
        U1 = np.einsum('jx,ocdx->ocjd', G1, w.astype(np.float32))
        w4 = U1.reshape(M, 128, M, 128, 4, 3)         # [m, co, k4, p, j, dy]
        out = np.transpose(w4, (0, 3, 4, 5, 2, 1))    # [m, p, j, dy, k4, co]
        out = out.reshape(M, 128, 12 * M, 128)
        return np.ascontiguousarray(out).astype(np.float16)

    wq = conv_w_wino(q_w)
    wk = conv_w_wino(kv_w[:C])
    wv = conv_w_wino(kv_w[C:])
    ch = np.arange(C)
    perm = (ch % cpg) * nh + ch // cpg                # proj input index per attn channel
    pwp = proj_w[:, :, 0, 0][:, perm]                 # [co, ch]
    pw = np.ascontiguousarray(pwp.T.reshape(M, 128, C)).astype(BF)
    biases = np.stack([q_b, kv_b[:C], kv_b[C:], proj_b]).astype(np.float32)
    return wq, wk, wv, pw, biases


_CACHE = {}


def _get_nc():
    if "nc" not in _CACHE:
        _CACHE["nc"] = build_nc(B=2, C=512, n_cores=8)
    return _CACHE["nc"]


def make_in_maps(x, q_w, q_b, kv_w, kv_b, proj_w, proj_b, n_cores=8):
    wq, wk, wv, pw, biases = prep_weights(
        np.asarray(q_w), np.asarray(q_b), np.asarray(kv_w), np.asarray(kv_b),
        np.asarray(proj_w), np.asarray(proj_b))
    x = np.asarray(x, dtype=np.float32).astype(np.float16)
    bpc = x.shape[0] // n_cores
    return [
        {"x": np.ascontiguousarray(x[i * bpc:(i + 1) * bpc]),
         "wq": wq, "wk": wk, "wv": wv, "pw": pw, "biases": biases}
        for i in range(n_cores)
    ]


def kernel(x, q_w, q_b, kv_w, kv_b, proj_w, proj_b):
    nc = _get_nc()
    in_maps = make_in_maps(x, q_w, q_b, kv_w, kv_b, proj_w, proj_b)
    res = run_bass_kernel_spmd(nc, in_maps, core_ids=list(range(8)))
    out = np.concatenate([res.results[i]["y"] for i in range(8)], axis=0)
    return out.astype(np.float32)



# revision 35
# speedup vs baseline: 1.0094x; 1.0029x over previous
"""Trainium2 Bass kernel for nn_Attention (conv-qkv spatial attention block).

Contract: kernel(**inputs) takes FULL unsharded inputs (B=16, C=512, H=W=64),
shards batch across 8 NeuronCores (2 images per core), runs one SPMD Bass
program, and returns the FULL output (fp32).

Math per image (reference):
  q  = conv3x3(x, q_w) + q_b                      # (C, H, W)
  kv = conv3x3(x, kv_w) + kv_b ; k, v = split(kv)
  per channel ch: attn = softmax(q_ch @ k_ch^T) ; o_ch = attn @ v_ch
  y  = conv1x1(perm(o), proj_w) + proj_b          # head/channel permutation
       (the permutation is folded into proj_w on the host)

Device implementation notes:
  - All three 3x3 convs use 1D Winograd F(2,3) along W (1.5x fewer MACs):
    weights are G-transformed on the host (U1[j,dy] = sum_dx G1[j,dx] w),
    the input B1^T transform runs once per image on DVE (4 tensor_tensor
    ops per ci-chunk over a zero-padded copy of x) and is shared by q/k/v;
    the GEMM contracts (ci, dy) per j-position in fp16 with fp32 PSUM;
    output pairs recombine from PSUM in fp32 (y0=M0+M1+M2+b, y1=M1-M2-M3+b)
    with the bias folded into an ACT evacuation of M1.
  - Conv internals (x, weights, V) are fp16 — same PE speed as bf16 but 8x
    finer mantissa, which suppresses the Winograd noise amplification that
    would otherwise break the peaked-softmax logits. Attention operands and
    all attention matmuls stay bf16 (exp values overflow fp16 range).
  - Per-channel attention operands are produced by DVE stream-transpose
    (32x32 blocks), giving a tiled layout where the spatial index lives on
    partitions mod 32 and attention runs as K=32 matmuls packed 4-wide on
    the PE array via tile_position quadrants.
  - softmax: exp in fp32 without max subtraction (logits bounded ~|75| < 88),
    row sums via a ones-matmul, one reciprocal + broadcast multiply.
  - Keep the bias/pw DMAs on the sync (HWDGE) queue: the SWDGE (gpsimd)
    queue corrupts the rearranged bias load on the execution backend (NaN).
"""

import numpy as np
import ml_dtypes

import concourse.bass as bass
import concourse.bacc as bacc
import concourse.mybir as mybir
import concourse.tile as tile
from concourse.bass_utils import run_bass_kernel_spmd

F32 = mybir.dt.float32
BF16 = mybir.dt.bfloat16
F16 = mybir.dt.float16
AF = mybir.ActivationFunctionType
BF = ml_dtypes.bfloat16

H = 64          # spatial height (attention over rows, contracting cols)
PW = 66         # padded row width
NPOS = H * H    # 4096 positions per image


def build_nc(B=2, C=512, n_cores=8, repeat=1, phases=("conv", "attn", "proj")):
    """Build the per-core Bass program. B = images per core.

    repeat > 1 emits the whole body multiple times (timing builds only).
    phases: drop "attn"/"proj" for timing-breakdown builds.
    """
    M = C // 128            # channel chunks (co chunks and ci chunks)
    nc = bacc.Bacc("TRN2", target_bir_lowering=False, debug=False,
                   num_devices=n_cores)

    x_d = nc.dram_tensor("x", [B, C, H, H], F16, kind="ExternalInput")
    wq_d = nc.dram_tensor("wq", [M, 128, 12 * M, 128], F16, kind="ExternalInput")
    wk_d = nc.dram_tensor("wk", [M, 128, 12 * M, 128], F16, kind="ExternalInput")
    # v conv uses 1D Winograd F(2,3) along W: 12 = 4 j-positions x 3 dy taps
    wv_d = nc.dram_tensor("wv", [M, 128, 12 * M, 128], F16, kind="ExternalInput")
    pw_d = nc.dram_tensor("pw", [M, 128, C], BF16, kind="ExternalInput")
    bias_d = nc.dram_tensor("biases", [4, C], F32, kind="ExternalInput")
    y_d = nc.dram_tensor("y", [B, C, H, H], F32, kind="ExternalOutput")

    with tile.TileContext(nc) as tc:
        _body(tc, nc, B, M, x_d, (wq_d, wk_d, wv_d), pw_d, bias_d, y_d,
              repeat=repeat, phases=phases)
    nc.compile()
    return nc


def _body(tc, nc, B, M, x_d, w_ds, pw_d, bias_d, y_d, repeat=1,
          phases=("conv", "attn", "proj")):
    from contextlib import ExitStack
    ctx = ExitStack()
    C = M * 128
    const = ctx.enter_context(tc.tile_pool(name="const", bufs=1))
    xpad_p = ctx.enter_context(tc.tile_pool(name="xpad", bufs=2))
    w_p = ctx.enter_context(tc.tile_pool(name="wconv", bufs=2))
    v_p = ctx.enter_context(tc.tile_pool(name="vwino", bufs=4))
    qkv_p = ctx.enter_context(tc.tile_pool(name="qkv", bufs=4))
    exp_p = ctx.enter_context(tc.tile_pool(name="exp", bufs=3))
    nt_p = ctx.enter_context(tc.tile_pool(name="normT", bufs=1))
    rc_p = ctx.enter_context(tc.tile_pool(name="recip", bufs=1))
    as_p = ctx.enter_context(tc.tile_pool(name="attns", bufs=1))
    acm_p = ctx.enter_context(tc.tile_pool(name="attncm", bufs=min(M, 4)))
    y_p = ctx.enter_context(tc.tile_pool(name="yout", bufs=2))
    st_p = ctx.enter_context(tc.tile_pool(name="stage", bufs=4))
    cp_ps = ctx.enter_context(tc.tile_pool(name="cpps", bufs=4, space="PSUM"))
    at_ps = ctx.enter_context(tc.tile_pool(name="atps", bufs=2, space="PSUM"))
    nm_ps = ctx.enter_context(tc.tile_pool(name="nmps", bufs=1, space="PSUM"))
    o2_ps = ctx.enter_context(tc.tile_pool(name="o2ps", bufs=1, space="PSUM"))

    # constants
    ones32 = const.tile([128, 32], BF16, tag="ones32")
    nc.gpsimd.memset(ones32[:, :], 1.0)
    # per-partition bias columns: col (ti*M + m) = bias[ti, m*128:(m+1)*128]
    bias_sb = const.tile([128, 4 * M], F32, tag="bias")
    nc.sync.dma_start(out=bias_sb[:, :],
                      in_=bias_d[:, :].rearrange("a (m p) -> p (a m)", p=128))
    pw_sb = const.tile([128, M * C], BF16, tag="pw")
    pw_loaded = [False]

    for b in [b for _ in range(repeat) for b in range(B)]:
        # ---- load x image b (fp16): zero-pad borders, then 1D Winograd
        # B1^T input transform along W (4 DVE ops per ci-chunk), shared by
        # the q/k/v GEMMs. V[j, r', tw] in fp16.
        SUB = mybir.AluOpType.subtract
        ADD = mybir.AluOpType.add
        vts = []
        for k4 in range(M):
            xp = xpad_p.tile([128, PW * PW], F16, tag="xpad")
            z = xp[:, :].rearrange("p (r c) -> p r c", c=PW)
            nc.gpsimd.memset(z[:, 0, :], 0.0)
            nc.gpsimd.memset(z[:, PW - 1, :], 0.0)
            nc.gpsimd.memset(z[:, :, 0], 0.0)
            nc.gpsimd.memset(z[:, :, PW - 1], 0.0)
            # x rows split in halves so transforms (and the first GEMM
            # groups, which read only low V rows) start before the full
            # image lands
            nc.sync.dma_start(out=z[:, 1:33, 1:H + 1],
                              in_=x_d[b, k4 * 128:(k4 + 1) * 128, 0:32, :])
            nc.sync.dma_start(out=z[:, 33:H + 1, 1:H + 1],
                              in_=x_d[b, k4 * 128:(k4 + 1) * 128, 32:H, :])
            vt = v_p.tile([128, 4 * PW * 32], F16, tag="vtile")
            vz = vt[:, :].rearrange("p (j r c) -> p j r c", j=4, r=PW)
            z2 = xp[:, :].rearrange("p (r c2 par) -> p r c2 par", par=2, c2=33)
            for r0, r1 in ((0, 33), (33, PW)):
                dA0 = z2[:, r0:r1, 0:32, 0]       # cp = 2tw
                dA1 = z2[:, r0:r1, 0:32, 1]       # cp = 2tw+1
                dB0 = z2[:, r0:r1, 1:33, 0]       # cp = 2tw+2
                dB1 = z2[:, r0:r1, 1:33, 1]       # cp = 2tw+3
                nc.vector.tensor_tensor(vz[:, 0, r0:r1], dA0, dB0, SUB)
                nc.vector.tensor_tensor(vz[:, 1, r0:r1], dA1, dB0, ADD)
                nc.vector.tensor_tensor(vz[:, 2, r0:r1], dB0, dA1, SUB)
                nc.vector.tensor_tensor(vz[:, 3, r0:r1], dA1, dB1, SUB)
            vts.append(vz)
            if k4 == 0:
                w_pre = w_p.tile([128, 12 * M * 128], F16, tag="wconv")
                for j in (1, 2, 0, 3):   # vgemm consumption order
                    nc.sync.dma_start(
                        out=w_pre[:, 3 * j * M * 128:3 * (j + 1) * M * 128],
                        in_=w_ds[0][0][:, 3 * j * M:3 * (j + 1) * M, :]
                        .rearrange("p a b -> p (a b)"))
            if k4 == 1 and not pw_loaded[0]:
                pw_loaded[0] = True
                for kk in range(M):
                    nc.sync.dma_start(out=pw_sb[:, kk * C:(kk + 1) * C],
                                      in_=pw_d[kk, :, :])

        acm = []  # attnout channel-major chunks for proj
        for m in range(M):
            outs = {}

            def vgemm(wt, j, rsl, csl, cpr):
                """Accumulate M_j = sum_{dy,ci} U1[j,dy] @ V[j] into a psum.
                rsl/csl slice V rows/tile-cols; cpr=True puts tw outer
                (col-major free order for the v staging)."""
                ps = cp_ps.tile([128, 512], F32, tag="cpps")
                for i2, (k4, dy) in enumerate(
                        (k, d) for k in range(M) for d in range(3)):
                    rhs = vts[k4][:, j, dy + rsl:dy + rsl + (64 if cpr else 16),
                                  csl:csl + (8 if cpr else 32)]
                    if cpr:
                        rhs = rhs.transpose([0, 2, 1])
                    nc.tensor.matmul(
                        ps[:, :],
                        wt[:, ((j * 3 + dy) * M + k4) * 128:
                           ((j * 3 + dy) * M + k4 + 1) * 128],
                        rhs, start=(i2 == 0), stop=(i2 == 3 * M - 1))
                return ps

            def ycombine(wt, tb, o_t, qt, cpr):
                """Winograd output recombine for one psum group -> 2 STs.
                y0 = M0+M1+M2+b ; y1 = M1-M2-M3+b (fp32 PSUM reads)."""
                rsl = 0 if cpr else qt * 16
                csl = qt * 8 if cpr else 0
                shp = (lambda t: t.rearrange("p (tw r) -> p tw r", tw=8)) \
                    if cpr else \
                    (lambda t: t.rearrange("p (r tw) -> p r tw", tw=32))
                m1 = vgemm(wt, 1, rsl, csl, cpr)
                ev1 = st_p.tile([128, 512], BF16, tag="stage")
                nc.scalar.activation(ev1[:, :], m1[:, :], AF.Identity, bias=tb)
                m2 = vgemm(wt, 2, rsl, csl, cpr)
                tws = st_p.tile([128, 512], BF16, tag="stage")
                twd = st_p.tile([128, 512], BF16, tag="stage")
                nc.vector.tensor_tensor(shp(tws[:, :]), shp(ev1[:, :]),
                                        shp(m2[:, :]), ADD)
                nc.vector.tensor_tensor(shp(twd[:, :]), shp(ev1[:, :]),
                                        shp(m2[:, :]), SUB)
                m0 = vgemm(wt, 0, rsl, csl, cpr)
                m3 = vgemm(wt, 3, rsl, csl, cpr)
                stg = st_p.tile([128, 1024], BF16, tag="stage2")
                if cpr:   # col-major: free = (c 16, r 64), c = 2tw+q
                    sz = stg[:, :].rearrange("p (tw q r) -> p tw q r",
                                             tw=8, q=2)
                    y0, y1 = sz[:, :, 0, :], sz[:, :, 1, :]
                else:     # row-major: free = (r 16, c 64), c = 2c2+q
                    sz = stg[:, :].rearrange("p (r c2 q) -> p r c2 q",
                                             q=2, c2=32)
                    y0, y1 = sz[:, :, :, 0], sz[:, :, :, 1]
                nc.vector.tensor_tensor(y0, shp(tws[:, :]), shp(m0[:, :]), ADD)
                nc.vector.tensor_tensor(y1, shp(twd[:, :]), shp(m3[:, :]), SUB)
                nc.vector.transpose(o_t[:, (2 * qt) * 512:(2 * qt + 1) * 512],
                                    stg[:, 0:512])
                nc.vector.transpose(o_t[:, (2 * qt + 1) * 512:
                                        (2 * qt + 2) * 512],
                                    stg[:, 512:1024])

            for ti, tname in enumerate(("q", "k", "v")):
                if m == 0 and ti == 0:
                    wt = w_pre
                else:
                    wt = w_p.tile([128, 12 * M * 128], F16, tag="wconv")
                    nc.sync.dma_start(
                        out=wt[:, :],
                        in_=w_ds[ti][m].rearrange("p a b -> p (a b)"))
                o_t = qkv_p.tile([128, NPOS], BF16, tag="qkv")
                tb = bias_sb[:, ti * M + m: ti * M + m + 1]
                for qt in range(4):
                    ycombine(wt, tb, o_t, qt, cpr=(tname == "v"))
                outs[tname] = o_t

            if "attn" not in phases:
                nc.gpsimd.dma_start(
                    out=y_d[b, m * 128:(m + 1) * 128].rearrange("p a b -> p (a b)"),
                    in_=outs["q"][:, :])
                continue

            # ---- attention for the 128 channels of chunk m
            o_q, o_k, o_v = outs["q"], outs["k"], outs["v"]
            a_s = as_p.tile([128, NPOS], BF16, tag="attns")
            kks = [o_k[cb * 32:(cb + 1) * 32, :].rearrange(
                "p (kid half c) -> p kid half c", half=2, c=32) for cb in range(4)]
            qqs = [o_q[cb * 32:(cb + 1) * 32, :].rearrange(
                "p (i half c) -> p i half c", half=2, c=32) for cb in range(4)]
            vvs = [o_v[cb * 32:(cb + 1) * 32, :].rearrange(
                "p (w half c) -> p w half c", half=2, c=32) for cb in range(4)]
            for qd in range(8):
                atp = at_ps.tile([128, 512], F32, tag="atps")
                # logits^T:  atp[cb*32+kappa, sl*128+kb*64+i] = sum_j k*q
                # cb innermost so consecutive MMs hit different PE quadrants
                for i1, (sl, kb, jb) in enumerate(
                        (s, k, j) for s in range(4) for k in range(2) for j in range(2)):
                    c = qd * 4 + sl
                    for cb in range(4):
                        nc.tensor.matmul(
                            atp[cb * 32:(cb + 1) * 32,
                                sl * 128 + kb * 64: sl * 128 + (kb + 1) * 64],
                            kks[cb][:, kb * 32:(kb + 1) * 32, jb, c],
                            qqs[cb][:, :, jb, c],
                            start=(i1 == 0), stop=(i1 == 15),
                            skip_group_check=True,
                            tile_position=(cb * 32, cb * 32))
                # exp (fp32 -> bf16), no max subtraction
                ex = exp_p.tile([128, 512], BF16, tag="exp")
                nc.scalar.activation(ex[:, :], atp[:, :], AF.Exp)
                # row sums (over kidx) via ones-matmul, replicated on 32 parts
                nmp = nm_ps.tile([128, 256], F32, tag="nmps")
                for kb in range(2):
                    for cb in range(4):
                        ee = ex[cb * 32:(cb + 1) * 32, :].rearrange(
                            "p (sl half i) -> p sl half i", half=2, i=64)
                        nc.tensor.matmul(
                            nmp[cb * 32:(cb + 1) * 32, :],
                            ones32[cb * 32:(cb + 1) * 32, :],
                            ee[:, :, kb, :],
                            start=(kb == 0), stop=(kb == 1),
                            skip_group_check=True,
                            tile_position=(cb * 32, cb * 32))
                nt = nt_p.tile([128, 256], F32, tag="normT")
                nc.vector.transpose(nt[:, :], nmp[:, :])
                rc = rc_p.tile([128, 8], F32, tag="recip")
                nc.vector.reciprocal(
                    rc[:, :], nt[:, :].rearrange("p (t c) -> p t c", c=32)[:, :, 0])
                # out2 = attn_exp^T' @ v   (unnormalized), K=32 chunks
                o2p = o2_ps.tile([128, 512], F32, tag="o2ps")
                for i2, (sl, ib, kb) in enumerate(
                        (s, i, k) for s in range(4) for i in range(2) for k in range(2)):
                    c = qd * 4 + sl
                    for cb in range(4):
                        nc.tensor.matmul(
                            o2p[cb * 32:(cb + 1) * 32,
                                sl * 128 + ib * 64: sl * 128 + (ib + 1) * 64],
                            ex[cb * 32:(cb + 1) * 32,
                               sl * 128 + kb * 64 + ib * 32:
                               sl * 128 + kb * 64 + ib * 32 + 32],
                            vvs[cb][:, :, kb, c],
                            start=(i2 == 0), stop=(i2 == 15),
                            skip_group_check=True,
                            tile_position=(cb * 32, cb * 32))
                # normalize + write into attnout_s (v-style layout), bf16
                in0 = o2p[:, :].rearrange("p (sl ib w) -> p sl ib w", ib=2, w=64)
                in1 = rc[:, :].rearrange("p (sl ib) -> p sl ib", ib=2)
                in1 = in1.unsqueeze(3).broadcast_to((128, 4, 2, 64))
                outap = a_s[:, :].rearrange("p (t c) -> p t c", c=32)
                outap = outap[:, :, qd * 4:qd * 4 + 4].rearrange(
                    "p (w ib) sl -> p w ib sl", ib=2).transpose([0, 3, 2, 1])
                nc.vector.tensor_tensor(outap, in0, in1, mybir.AluOpType.mult)
            # back-transpose to channel-major (column-major positions)
            a_cm = acm_p.tile([128, NPOS], BF16, tag="attncm")
            nc.vector.transpose(a_cm[:, :], a_s[:, :])
            acm.append(a_cm)

        if "attn" not in phases:
            continue
        if "proj" not in phases:
            for m in range(M):
                nc.gpsimd.dma_start(
                    out=y_d[b, m * 128:(m + 1) * 128].rearrange("p a b -> p (a b)"),
                    in_=acm[m][:, :])
            del acm
            continue

        # ---- proj (1x1 conv with permuted weights) + bias, row-major out.
        # y-writes batched in pairs of psum groups (one 512 KB DMA per 16
        # output rows) to halve the per-DMA overhead on the DMA engines.
        for mo in range(M):
            for n2 in range(4):
                yt = y_p.tile([128, NPOS // 4], F32, tag="yout")
                for half in range(2):
                    n = n2 * 2 + half
                    psum = cp_ps.tile([128, 512], F32, tag="cpps")
                    for k4 in range(M):
                        rhs = acm[k4][:, :].rearrange("p (w i) -> p w i", i=64)
                        rhs = rhs[:, :, n * 8:(n + 1) * 8].transpose([0, 2, 1])
                        nc.tensor.matmul(
                            psum[:, :],
                            pw_sb[:, k4 * C + mo * 128: k4 * C + (mo + 1) * 128],
                            rhs, start=(k4 == 0), stop=(k4 == M - 1))
                    nc.scalar.activation(
                        yt[:, half * 512:(half + 1) * 512], psum[:, :],
                        AF.Identity,
                        bias=bias_sb[:, 3 * M + mo: 3 * M + mo + 1])
                nc.sync.dma_start(
                    out=y_d[b, mo * 128:(mo + 1) * 128,
                            n2 * 16:(n2 + 1) * 16, :],
                    in_=yt[:, :])
        del acm
    ctx.close()


def prep_weights(q_w, q_b, kv_w, kv_b, proj_w, proj_b, C=512):
    """Host-side weight re-layouts (numpy, bf16)."""
    M = C // 128
    nh = 16
    cpg = C // nh

    def conv_w(w):
        # w[co, ci, dy, dx] -> [m, p(ci%128), t(=dy*3+dx), k4, co] flat
        w4 = w.reshape(M, 128, M, 128, 3, 3)          # [m, co, k4, p, dy, dx]
        out = np.transpose(w4, (0, 3, 4, 5, 2, 1))    # [m, p, dy, dx, k4, co]
        out = out.reshape(M, 128, 9 * M, 128)
        return np.ascontiguousarray(out).astype(BF)

    def conv_w_wino(w):
        # 1D Winograd F(2,3) along W: U1[j,dy,o,c] = sum_dx G1[j,dx] w[o,c,dy,dx]
        # layout [m, p(ci%128), t'(=j*3+dy), k4, co] flat
        G1 = np.array([[1, 0, 0], [.5, .5, .5], [.5, -.5, .5], [0, 0, 1]],
                      np.float32)
        U1 = np.einsum('jx,ocdx->ocjd', G1, w.astype(np.float32))
        w4 = U1.reshape(M, 128, M, 128, 4, 3)         # [m, co, k4, p, j, dy]
        out = np.transpose(w4, (0, 3, 4, 5, 2, 1))    # [m, p, j, dy, k4, co]
        out = out.reshape(M, 128, 12 * M, 128)
        return np.ascontiguousarray(out).astype(np.float16)

    wq = conv_w_wino(q_w)
    wk = conv_w_wino(kv_w[:C])
    wv = conv_w_wino(kv_w[C:])
    ch = np.arange(C)
    perm = (ch % cpg) * nh + ch // cpg                # proj input index per attn channel
    pwp = proj_w[:, :, 0, 0][:, perm]                 # [co, ch]
    pw = np.ascontiguousarray(pwp.T.reshape(M, 128, C)).astype(BF)
    biases = np.stack([q_b, kv_b[:C], kv_b[C:], proj_b]).astype(np.float32)
    return wq, wk, wv, pw, biases


_CACHE = {}


def _get_nc():
    if "nc" not in _CACHE:
        _CACHE["nc"] = build_nc(B=2, C=512, n_cores=8)
    return _CACHE["nc"]


def make_in_maps(x, q_w, q_b, kv_w, kv_b, proj_w, proj_b, n_cores=8):
    wq, wk, wv, pw, biases = prep_weights(
        np.asarray(q_w), np.asarray(q_b), np.asarray(kv_w), np.asarray(kv_b),
        np.asarray(proj_w), np.asarray(proj_b))
    x = np.asarray(x, dtype=np.float32).astype(np.float16)
    bpc = x.shape[0] // n_cores
    return [
        {"x": np.ascontiguousarray(x[i * bpc:(i + 1) * bpc]),
         "wq": wq, "wk": wk, "wv": wv, "pw": pw, "biases": biases}
        for i in range(n_cores)
    ]


def kernel(x, q_w, q_b, kv_w, kv_b, proj_w, proj_b):
    nc = _get_nc()
    in_maps = make_in_maps(x, q_w, q_b, kv_w, kv_b, proj_w, proj_b)
    res = run_bass_kernel_spmd(nc, in_maps, core_ids=list(range(8)))
    out = np.concatenate([res.results[i]["y"] for i in range(8)], axis=0)
    return out.astype(np.float32)



# revision 37
# speedup vs baseline: 1.0099x; 1.0005x over previous
"""Trainium2 Bass kernel for nn_Attention (conv-qkv spatial attention block).

Contract: kernel(**inputs) takes FULL unsharded inputs (B=16, C=512, H=W=64),
shards batch across 8 NeuronCores (2 images per core), runs one SPMD Bass
program, and returns the FULL output (fp32).

Math per image (reference):
  q  = conv3x3(x, q_w) + q_b                      # (C, H, W)
  kv = conv3x3(x, kv_w) + kv_b ; k, v = split(kv)
  per channel ch: attn = softmax(q_ch @ k_ch^T) ; o_ch = attn @ v_ch
  y  = conv1x1(perm(o), proj_w) + proj_b          # head/channel permutation
       (the permutation is folded into proj_w on the host)

Device implementation notes:
  - All three 3x3 convs use 1D Winograd F(2,3) along W (1.5x fewer MACs):
    weights are G-transformed on the host (U1[j,dy] = sum_dx G1[j,dx] w),
    the input B1^T transform runs once per image on DVE (4 tensor_tensor
    ops per ci-chunk over a zero-padded copy of x) and is shared by q/k/v;
    the GEMM contracts (ci, dy) per j-position in fp16 with fp32 PSUM;
    output pairs recombine from PSUM in fp32 (y0=M0+M1+M2+b, y1=M1-M2-M3+b)
    with the bias folded into an ACT evacuation of M1.
  - Conv internals (x, weights, V) are fp16 — same PE speed as bf16 but 8x
    finer mantissa, which suppresses the Winograd noise amplification that
    would otherwise break the peaked-softmax logits. Attention operands and
    all attention matmuls stay bf16 (exp values overflow fp16 range).
  - Per-channel attention operands are produced by DVE stream-transpose
    (32x32 blocks), giving a tiled layout where the spatial index lives on
    partitions mod 32 and attention runs as K=32 matmuls packed 4-wide on
    the PE array via tile_position quadrants.
  - softmax: exp in fp32 without max subtraction (logits bounded ~|75| < 88),
    row sums via a ones-matmul, one reciprocal + broadcast multiply.
  - Keep the bias/pw DMAs on the sync (HWDGE) queue: the SWDGE (gpsimd)
    queue corrupts the rearranged bias load on the execution backend (NaN).
"""

import numpy as np
import ml_dtypes

import concourse.bass as bass
import concourse.bacc as bacc
import concourse.mybir as mybir
import concourse.tile as tile
from concourse.bass_utils import run_bass_kernel_spmd

F32 = mybir.dt.float32
BF16 = mybir.dt.bfloat16
F16 = mybir.dt.float16
AF = mybir.ActivationFunctionType
BF = ml_dtypes.bfloat16

H = 64          # spatial height (attention over rows, contracting cols)
PW = 66         # padded row width
NPOS = H * H    # 4096 positions per image


def build_nc(B=2, C=512, n_cores=8, repeat=1, phases=("conv", "attn", "proj")):
    """Build the per-core Bass program. B = images per core.

    repeat > 1 emits the whole body multiple times (timing builds only).
    phases: drop "attn"/"proj" for timing-breakdown builds.
    """
    M = C // 128            # channel chunks (co chunks and ci chunks)
    nc = bacc.Bacc("TRN2", target_bir_lowering=False, debug=False,
                   num_devices=n_cores)

    x_d = nc.dram_tensor("x", [B, C, H, H], F16, kind="ExternalInput")
    wq_d = nc.dram_tensor("wq", [M, 128, 12 * M, 128], F16, kind="ExternalInput")
    wk_d = nc.dram_tensor("wk", [M, 128, 12 * M, 128], F16, kind="ExternalInput")
    # v conv uses 1D Winograd F(2,3) along W: 12 = 4 j-positions x 3 dy taps
    wv_d = nc.dram_tensor("wv", [M, 128, 12 * M, 128], F16, kind="ExternalInput")
    pw_d = nc.dram_tensor("pw", [M, 128, C], BF16, kind="ExternalInput")
    bias_d = nc.dram_tensor("biases", [4, C], F32, kind="ExternalInput")
    y_d = nc.dram_tensor("y", [B, C, H, H], F32, kind="ExternalOutput")

    with tile.TileContext(nc) as tc:
        _body(tc, nc, B, M, x_d, (wq_d, wk_d, wv_d), pw_d, bias_d, y_d,
              repeat=repeat, phases=phases)
    nc.compile()
    return nc


def _body(tc, nc, B, M, x_d, w_ds, pw_d, bias_d, y_d, repeat=1,
          phases=("conv", "attn", "proj")):
    from contextlib import ExitStack
    ctx = ExitStack()
    C = M * 128
    const = ctx.enter_context(tc.tile_pool(name="const", bufs=1))
    xpad_p = ctx.enter_context(tc.tile_pool(name="xpad", bufs=2))
    w_p = ctx.enter_context(tc.tile_pool(name="wconv", bufs=2))
    v_p = ctx.enter_context(tc.tile_pool(name="vwino", bufs=4))
    qkv_p = ctx.enter_context(tc.tile_pool(name="qkv", bufs=4))
    exp_p = ctx.enter_context(tc.tile_pool(name="exp", bufs=3))
    nt_p = ctx.enter_context(tc.tile_pool(name="normT", bufs=1))
    rc_p = ctx.enter_context(tc.tile_pool(name="recip", bufs=1))
    as_p = ctx.enter_context(tc.tile_pool(name="attns", bufs=1))
    acm_p = ctx.enter_context(tc.tile_pool(name="attncm", bufs=min(M, 4)))
    y_p = ctx.enter_context(tc.tile_pool(name="yout", bufs=2))
    st_p = ctx.enter_context(tc.tile_pool(name="stage", bufs=4))
    cp_ps = ctx.enter_context(tc.tile_pool(name="cpps", bufs=4, space="PSUM"))
    at_ps = ctx.enter_context(tc.tile_pool(name="atps", bufs=2, space="PSUM"))
    nm_ps = ctx.enter_context(tc.tile_pool(name="nmps", bufs=1, space="PSUM"))
    o2_ps = ctx.enter_context(tc.tile_pool(name="o2ps", bufs=1, space="PSUM"))

    # constants
    ones32 = const.tile([128, 32], BF16, tag="ones32")
    nc.gpsimd.memset(ones32[:, :], 1.0)
    # per-partition bias columns: col (ti*M + m) = bias[ti, m*128:(m+1)*128]
    bias_sb = const.tile([128, 4 * M], F32, tag="bias")
    nc.sync.dma_start(out=bias_sb[:, :],
                      in_=bias_d[:, :].rearrange("a (m p) -> p (a m)", p=128))
    pw_sb = const.tile([128, M * C], BF16, tag="pw")
    pw_loaded = [False]

    def load_xpad(b_, k4_):
        """memset borders + row-halved x DMA for one ci-chunk."""
        xp = xpad_p.tile([128, PW * PW], F16, tag="xpad")
        z = xp[:, :].rearrange("p (r c) -> p r c", c=PW)
        nc.gpsimd.memset(z[:, 0, :], 0.0)
        nc.gpsimd.memset(z[:, PW - 1, :], 0.0)
        nc.gpsimd.memset(z[:, :, 0], 0.0)
        nc.gpsimd.memset(z[:, :, PW - 1], 0.0)
        nc.sync.dma_start(out=z[:, 1:33, 1:H + 1],
                          in_=x_d[b_, k4_ * 128:(k4_ + 1) * 128, 0:32, :])
        nc.sync.dma_start(out=z[:, 33:H + 1, 1:H + 1],
                          in_=x_d[b_, k4_ * 128:(k4_ + 1) * 128, 32:H, :])
        return xp

    xpre = {}
    xpre_b = [None]

    bseq = [b for _ in range(repeat) for b in range(B)]
    for bi, b in enumerate(bseq):
        # ---- load x image b (fp16): zero-pad borders, then 1D Winograd
        # B1^T input transform along W (4 DVE ops per ci-chunk), shared by
        # the q/k/v GEMMs. V[j, r', tw] in fp16.
        SUB = mybir.AluOpType.subtract
        ADD = mybir.AluOpType.add
        vts = []
        for k4 in range(M):
            xp = xpre.pop(k4, None) if b == xpre_b[0] else None
            if xp is None:
                xp = load_xpad(b, k4)
            z = xp[:, :].rearrange("p (r c) -> p r c", c=PW)
            vt = v_p.tile([128, 4 * PW * 32], F16, tag="vtile")
            vz = vt[:, :].rearrange("p (j r c) -> p j r c", j=4, r=PW)
            z2 = xp[:, :].rearrange("p (r c2 par) -> p r c2 par", par=2, c2=33)
            rsplit = ((0, 19), (19, 33), (33, PW)) if k4 == 0 else \
                ((0, 33), (33, PW))
            for r0, r1 in rsplit:
                dA0 = z2[:, r0:r1, 0:32, 0]       # cp = 2tw
                dA1 = z2[:, r0:r1, 0:32, 1]       # cp = 2tw+1
                dB0 = z2[:, r0:r1, 1:33, 0]       # cp = 2tw+2
                dB1 = z2[:, r0:r1, 1:33, 1]       # cp = 2tw+3
                nc.vector.tensor_tensor(vz[:, 0, r0:r1], dA0, dB0, SUB)
                nc.vector.tensor_tensor(vz[:, 1, r0:r1], dA1, dB0, ADD)
                nc.vector.tensor_tensor(vz[:, 2, r0:r1], dB0, dA1, SUB)
                nc.vector.tensor_tensor(vz[:, 3, r0:r1], dA1, dB1, SUB)
            vts.append(vz)
            if k4 == 0:
                w_pre = w_p.tile([128, 12 * M * 128], F16, tag="wconv")
                for j in (1, 2, 0, 3):   # vgemm consumption order
                    nc.sync.dma_start(
                        out=w_pre[:, 3 * j * M * 128:3 * (j + 1) * M * 128],
                        in_=w_ds[0][0][:, 3 * j * M:3 * (j + 1) * M, :]
                        .rearrange("p a b -> p (a b)"))
            if k4 == 1 and not pw_loaded[0]:
                pw_loaded[0] = True
                for kk in range(M):
                    nc.sync.dma_start(out=pw_sb[:, kk * C:(kk + 1) * C],
                                      in_=pw_d[kk, :, :])

        acm = []  # attnout channel-major chunks for proj
        for m in range(M):
            outs = {}

            def vgemm(wt, j, rsl, csl, cpr):
                """Accumulate M_j = sum_{dy,ci} U1[j,dy] @ V[j] into a psum.
                rsl/csl slice V rows/tile-cols; cpr=True puts tw outer
                (col-major free order for the v staging)."""
                ps = cp_ps.tile([128, 512], F32, tag="cpps")
                for i2, (k4, dy) in enumerate(
                        (k, d) for k in range(M) for d in range(3)):
                    rhs = vts[k4][:, j, dy + rsl:dy + rsl + (64 if cpr else 16),
                                  csl:csl + (8 if cpr else 32)]
                    if cpr:
                        rhs = rhs.transpose([0, 2, 1])
                    nc.tensor.matmul(
                        ps[:, :],
                        wt[:, ((j * 3 + dy) * M + k4) * 128:
                           ((j * 3 + dy) * M + k4 + 1) * 128],
                        rhs, start=(i2 == 0), stop=(i2 == 3 * M - 1))
                return ps

            def ycombine(wt, tb, o_t, qt, cpr):
                """Winograd output recombine for one psum group -> 2 STs.
                y0 = M0+M1+M2+b ; y1 = M1-M2-M3+b (fp32 PSUM reads)."""
                rsl = 0 if cpr else qt * 16
                csl = qt * 8 if cpr else 0
                shp = (lambda t: t.rearrange("p (tw r) -> p tw r", tw=8)) \
                    if cpr else \
                    (lambda t: t.rearrange("p (r tw) -> p r tw", tw=32))
                m1 = vgemm(wt, 1, rsl, csl, cpr)
                ev1 = st_p.tile([128, 512], BF16, tag="stage")
                nc.scalar.activation(ev1[:, :], m1[:, :], AF.Identity, bias=tb)
                m2 = vgemm(wt, 2, rsl, csl, cpr)
                tws = st_p.tile([128, 512], BF16, tag="stage")
                twd = st_p.tile([128, 512], BF16, tag="stage")
                nc.vector.tensor_tensor(shp(tws[:, :]), shp(ev1[:, :]),
                                        shp(m2[:, :]), ADD)
                nc.vector.tensor_tensor(shp(twd[:, :]), shp(ev1[:, :]),
                                        shp(m2[:, :]), SUB)
                m0 = vgemm(wt, 0, rsl, csl, cpr)
                m3 = vgemm(wt, 3, rsl, csl, cpr)
                stg = st_p.tile([128, 1024], BF16, tag="stage2")
                if cpr:   # col-major: free = (c 16, r 64), c = 2tw+q
                    sz = stg[:, :].rearrange("p (tw q r) -> p tw q r",
                                             tw=8, q=2)
                    y0, y1 = sz[:, :, 0, :], sz[:, :, 1, :]
                else:     # row-major: free = (r 16, c 64), c = 2c2+q
                    sz = stg[:, :].rearrange("p (r c2 q) -> p r c2 q",
                                             q=2, c2=32)
                    y0, y1 = sz[:, :, :, 0], sz[:, :, :, 1]
                nc.vector.tensor_tensor(y0, shp(tws[:, :]), shp(m0[:, :]), ADD)
                nc.vector.tensor_tensor(y1, shp(twd[:, :]), shp(m3[:, :]), SUB)
                nc.vector.transpose(o_t[:, (2 * qt) * 512:(2 * qt + 1) * 512],
                                    stg[:, 0:512])
                nc.vector.transpose(o_t[:, (2 * qt + 1) * 512:
                                        (2 * qt + 2) * 512],
                                    stg[:, 512:1024])

            for ti, tname in enumerate(("q", "k", "v")):
                if m == 0 and ti == 0:
                    wt = w_pre
                else:
                    wt = w_p.tile([128, 12 * M * 128], F16, tag="wconv")
                    nc.sync.dma_start(
                        out=wt[:, :],
                        in_=w_ds[ti][m].rearrange("p a b -> p (a b)"))
                o_t = qkv_p.tile([128, NPOS], BF16, tag="qkv")
                tb = bias_sb[:, ti * M + m: ti * M + m + 1]
                for qt in range(4):
                    ycombine(wt, tb, o_t, qt, cpr=(tname == "v"))
                outs[tname] = o_t

            if "attn" not in phases:
                nc.gpsimd.dma_start(
                    out=y_d[b, m * 128:(m + 1) * 128].rearrange("p a b -> p (a b)"),
                    in_=outs["q"][:, :])
                continue

            # ---- attention for the 128 channels of chunk m
            o_q, o_k, o_v = outs["q"], outs["k"], outs["v"]
            a_s = as_p.tile([128, NPOS], BF16, tag="attns")
            kks = [o_k[cb * 32:(cb + 1) * 32, :].rearrange(
                "p (kid half c) -> p kid half c", half=2, c=32) for cb in range(4)]
            qqs = [o_q[cb * 32:(cb + 1) * 32, :].rearrange(
                "p (i half c) -> p i half c", half=2, c=32) for cb in range(4)]
            vvs = [o_v[cb * 32:(cb + 1) * 32, :].rearrange(
                "p (w half c) -> p w half c", half=2, c=32) for cb in range(4)]
            for qd in range(8):
                atp = at_ps.tile([128, 512], F32, tag="atps")
                # logits^T:  atp[cb*32+kappa, sl*128+kb*64+i] = sum_j k*q
                # cb innermost so consecutive MMs hit different PE quadrants
                for i1, (sl, kb, jb) in enumerate(
                        (s, k, j) for s in range(4) for k in range(2) for j in range(2)):
                    c = qd * 4 + sl
                    for cb in range(4):
                        nc.tensor.matmul(
                            atp[cb * 32:(cb + 1) * 32,
                                sl * 128 + kb * 64: sl * 128 + (kb + 1) * 64],
                            kks[cb][:, kb * 32:(kb + 1) * 32, jb, c],
                            qqs[cb][:, :, jb, c],
                            start=(i1 == 0), stop=(i1 == 15),
                            skip_group_check=True,
                            tile_position=(cb * 32, cb * 32))
                # exp (fp32 -> bf16), no max subtraction
                ex = exp_p.tile([128, 512], BF16, tag="exp")
                nc.scalar.activation(ex[:, :], atp[:, :], AF.Exp)
                # row sums (over kidx) via ones-matmul, replicated on 32 parts
                nmp = nm_ps.tile([128, 256], F32, tag="nmps")
                for kb in range(2):
                    for cb in range(4):
                        ee = ex[cb * 32:(cb + 1) * 32, :].rearrange(
                            "p (sl half i) -> p sl half i", half=2, i=64)
                        nc.tensor.matmul(
                            nmp[cb * 32:(cb + 1) * 32, :],
                            ones32[cb * 32:(cb + 1) * 32, :],
                            ee[:, :, kb, :],
                            start=(kb == 0), stop=(kb == 1),
                            skip_group_check=True,
                            tile_position=(cb * 32, cb * 32))
                nt = nt_p.tile([128, 256], F32, tag="normT")
                nc.vector.transpose(nt[:, :], nmp[:, :])
                rc = rc_p.tile([128, 8], F32, tag="recip")
                nc.vector.reciprocal(
                    rc[:, :], nt[:, :].rearrange("p (t c) -> p t c", c=32)[:, :, 0])
                # out2 = attn_exp^T' @ v   (unnormalized), K=32 chunks
                o2p = o2_ps.tile([128, 512], F32, tag="o2ps")
                for i2, (sl, ib, kb) in enumerate(
                        (s, i, k) for s in range(4) for i in range(2) for k in range(2)):
                    c = qd * 4 + sl
                    for cb in range(4):
                        nc.tensor.matmul(
                            o2p[cb * 32:(cb + 1) * 32,
                                sl * 128 + ib * 64: sl * 128 + (ib + 1) * 64],
                            ex[cb * 32:(cb + 1) * 32,
                               sl * 128 + kb * 64 + ib * 32:
                               sl * 128 + kb * 64 + ib * 32 + 32],
                            vvs[cb][:, :, kb, c],
                            start=(i2 == 0), stop=(i2 == 15),
                            skip_group_check=True,
                            tile_position=(cb * 32, cb * 32))
                # normalize + write into attnout_s (v-style layout), bf16
                in0 = o2p[:, :].rearrange("p (sl ib w) -> p sl ib w", ib=2, w=64)
                in1 = rc[:, :].rearrange("p (sl ib) -> p sl ib", ib=2)
                in1 = in1.unsqueeze(3).broadcast_to((128, 4, 2, 64))
                outap = a_s[:, :].rearrange("p (t c) -> p t c", c=32)
                outap = outap[:, :, qd * 4:qd * 4 + 4].rearrange(
                    "p (w ib) sl -> p w ib sl", ib=2).transpose([0, 3, 2, 1])
                nc.vector.tensor_tensor(outap, in0, in1, mybir.AluOpType.mult)
            # back-transpose to channel-major (column-major positions)
            a_cm = acm_p.tile([128, NPOS], BF16, tag="attncm")
            nc.vector.transpose(a_cm[:, :], a_s[:, :])
            acm.append(a_cm)

        if "attn" not in phases:
            continue
        if "proj" not in phases:
            for m in range(M):
                nc.gpsimd.dma_start(
                    out=y_d[b, m * 128:(m + 1) * 128].rearrange("p a b -> p (a b)"),
                    in_=acm[m][:, :])
            del acm
            continue

        if bi + 1 < len(bseq):
            xpre_b[0] = bseq[bi + 1]
            for k4 in (0, 1):
                xpre[k4] = load_xpad(bseq[bi + 1], k4)

        # ---- proj (1x1 conv with permuted weights) + bias, row-major out.
        # y-writes batched in pairs of psum groups (one 512 KB DMA per 16
        # output rows) to halve the per-DMA overhead on the DMA engines.
        for mo in range(M):
            for n2 in range(4):
                yt = y_p.tile([128, NPOS // 4], F32, tag="yout")
                for half in range(2):
                    n = n2 * 2 + half
                    psum = cp_ps.tile([128, 512], F32, tag="cpps")
                    for k4 in range(M):
                        rhs = acm[k4][:, :].rearrange("p (w i) -> p w i", i=64)
                        rhs = rhs[:, :, n * 8:(n + 1) * 8].transpose([0, 2, 1])
                        nc.tensor.matmul(
                            psum[:, :],
                            pw_sb[:, k4 * C + mo * 128: k4 * C + (mo + 1) * 128],
                            rhs, start=(k4 == 0), stop=(k4 == M - 1))
                    nc.scalar.activation(
                        yt[:, half * 512:(half + 1) * 512], psum[:, :],
                        AF.Identity,
                        bias=bias_sb[:, 3 * M + mo: 3 * M + mo + 1])
                if b == B - 1 and mo == M - 1 and n2 == 3:
                    nc.sync.dma_start(
                        out=y_d[b, mo * 128:(mo + 1) * 128, 48:56, :],
                        in_=yt[:, 0:512])
                    nc.sync.dma_start(
                        out=y_d[b, mo * 128:(mo + 1) * 128, 56:64, :],
                        in_=yt[:, 512:1024])
                else:
                    nc.sync.dma_start(
                        out=y_d[b, mo * 128:(mo + 1) * 128,
                                n2 * 16:(n2 + 1) * 16, :],
                        in_=yt[:, :])
        del acm
    ctx.close()


def prep_weights(q_w, q_b, kv_w, kv_b, proj_w, proj_b, C=512):
    """Host-side weight re-layouts (numpy, bf16)."""
    M = C // 128
    nh = 16
    cpg = C // nh

    def conv_w(w):
        # w[co, ci, dy, dx] -> [m, p(ci%128), t(=dy*3+dx), k4, co] flat
        w4 = w.reshape(M, 128, M, 128, 3, 3)          # [m, co, k4, p, dy, dx]
        out = np.transpose(w4, (0, 3, 4, 5, 2, 1))    # [m, p, dy, dx, k4, co]
        out = out.reshape(M, 128, 9 * M, 128)
        return np.ascontiguousarray(out).astype(BF)

    def conv_w_wino(w):
        # 1D Winograd F(2,3) along W: U1[j,dy,o,c] = sum_dx G1[j,dx] w[o,c,dy,dx]
        # layout [m, p(ci%128), t'(=j*3+dy), k4, co] flat
        G1 = np.array([[1, 0, 0], [.5, .5, .5], [.5, -.5, .5], [0, 0, 1]],
                      np.float32)
        U1 = np.einsum('jx,ocdx->ocjd', G1, w.astype(np.float32))
        w4 = U1.reshape(M, 128, M, 128, 4, 3)         # [m, co, k4, p, j, dy]
        out = np.transpose(w4, (0, 3, 4, 5, 2, 1))    # [m, p, j, dy, k4, co]
        out = out.reshape(M, 128, 12 * M, 128)
        return np.ascontiguousarray(out).astype(np.float16)

    wq = conv_w_wino(q_w)
    wk = conv_w_wino(kv_w[:C])
    wv = conv_w_wino(kv_w[C:])
    ch = np.arange(C)
    perm = (ch % cpg) * nh + ch // cpg                # proj input index per attn channel
    pwp = proj_w[:, :, 0, 0][:, perm]                 # [co, ch]
    pw = np.ascontiguousarray(pwp.T.reshape(M, 128, C)).astype(BF)
    biases = np.stack([q_b, kv_b[:C], kv_b[C:], proj_b]).astype(np.float32)
    return wq, wk, wv, pw, biases


_CACHE = {}


def _get_nc():
    if "nc" not in _CACHE:
        _CACHE["nc"] = build_nc(B=2, C=512, n_cores=8)
    return _CACHE["nc"]


def make_in_maps(x, q_w, q_b, kv_w, kv_b, proj_w, proj_b, n_cores=8):
    wq, wk, wv, pw, biases = prep_weights(
        np.asarray(q_w), np.asarray(q_b), np.asarray(kv_w), np.asarray(kv_b),
        np.asarray(proj_w), np.asarray(proj_b))
    x = np.asarray(x, dtype=np.float32).astype(np.float16)
    bpc = x.shape[0] // n_cores
    return [
        {"x": np.ascontiguousarray(x[i * bpc:(i + 1) * bpc]),
         "wq": wq, "wk": wk, "wv": wv, "pw": pw, "biases": biases}
        for i in range(n_cores)
    ]


def kernel(x, q_w, q_b, kv_w, kv_b, proj_w, proj_b):
    nc = _get_nc()
    in_maps = make_in_maps(x, q_w, q_b, kv_w, kv_b, proj_w, proj_b)
    res = run_bass_kernel_spmd(nc, in_maps, core_ids=list(range(8)))
    out = np.concatenate([res.results[i]["y"] for i in range(8)], axis=0)
    return out.astype(np.float32)



# revision 38
# speedup vs baseline: 1.0105x; 1.0006x over previous
"""Trainium2 Bass kernel for nn_Attention (conv-qkv spatial attention block).

Contract: kernel(**inputs) takes FULL unsharded inputs (B=16, C=512, H=W=64),
shards batch across 8 NeuronCores (2 images per core), runs one SPMD Bass
program, and returns the FULL output (fp32).

Math per image (reference):
  q  = conv3x3(x, q_w) + q_b                      # (C, H, W)
  kv = conv3x3(x, kv_w) + kv_b ; k, v = split(kv)
  per channel ch: attn = softmax(q_ch @ k_ch^T) ; o_ch = attn @ v_ch
  y  = conv1x1(perm(o), proj_w) + proj_b          # head/channel permutation
       (the permutation is folded into proj_w on the host)

Device implementation notes:
  - All three 3x3 convs use 1D Winograd F(2,3) along W (1.5x fewer MACs):
    weights are G-transformed on the host (U1[j,dy] = sum_dx G1[j,dx] w),
    the input B1^T transform runs once per image on DVE (4 tensor_tensor
    ops per ci-chunk over a zero-padded copy of x) and is shared by q/k/v;
    the GEMM contracts (ci, dy) per j-position in fp16 with fp32 PSUM;
    output pairs recombine from PSUM in fp32 (y0=M0+M1+M2+b, y1=M1-M2-M3+b)
    with the bias folded into an ACT evacuation of M1.
  - Conv internals (x, weights, V) are fp16 — same PE speed as bf16 but 8x
    finer mantissa, which suppresses the Winograd noise amplification that
    would otherwise break the peaked-softmax logits. Attention operands and
    all attention matmuls stay bf16 (exp values overflow fp16 range).
  - Per-channel attention operands are produced by DVE stream-transpose
    (32x32 blocks), giving a tiled layout where the spatial index lives on
    partitions mod 32 and attention runs as K=32 matmuls packed 4-wide on
    the PE array via tile_position quadrants.
  - softmax: exp in fp32 without max subtraction (logits bounded ~|75| < 88),
    row sums via a ones-matmul, one reciprocal + broadcast multiply.
  - Keep the bias/pw DMAs on the sync (HWDGE) queue: the SWDGE (gpsimd)
    queue corrupts the rearranged bias load on the execution backend (NaN).
"""

import numpy as np
import ml_dtypes

import concourse.bass as bass
import concourse.bacc as bacc
import concourse.mybir as mybir
import concourse.tile as tile
from concourse.bass_utils import run_bass_kernel_spmd

F32 = mybir.dt.float32
BF16 = mybir.dt.bfloat16
F16 = mybir.dt.float16
AF = mybir.ActivationFunctionType
BF = ml_dtypes.bfloat16

H = 64          # spatial height (attention over rows, contracting cols)
PW = 66         # padded row width
NPOS = H * H    # 4096 positions per image


def build_nc(B=2, C=512, n_cores=8, repeat=1, phases=("conv", "attn", "proj")):
    """Build the per-core Bass program. B = images per core.

    repeat > 1 emits the whole body multiple times (timing builds only).
    phases: drop "attn"/"proj" for timing-breakdown builds.
    """
    M = C // 128            # channel chunks (co chunks and ci chunks)
    nc = bacc.Bacc("TRN2", target_bir_lowering=False, debug=False,
                   num_devices=n_cores)

    x_d = nc.dram_tensor("x", [B, C, H, H], F16, kind="ExternalInput")
    wq_d = nc.dram_tensor("wq", [M, 128, 12 * M, 128], F16, kind="ExternalInput")
    wk_d = nc.dram_tensor("wk", [M, 128, 12 * M, 128], F16, kind="ExternalInput")
    # v conv uses 1D Winograd F(2,3) along W: 12 = 4 j-positions x 3 dy taps
    wv_d = nc.dram_tensor("wv", [M, 128, 12 * M, 128], F16, kind="ExternalInput")
    pw_d = nc.dram_tensor("pw", [M, 128, C], BF16, kind="ExternalInput")
    bias_d = nc.dram_tensor("biases", [4, C], F32, kind="ExternalInput")
    y_d = nc.dram_tensor("y", [B, C, H, H], F32, kind="ExternalOutput")

    with tile.TileContext(nc) as tc:
        _body(tc, nc, B, M, x_d, (wq_d, wk_d, wv_d), pw_d, bias_d, y_d,
              repeat=repeat, phases=phases)
    nc.compile()
    return nc


def _body(tc, nc, B, M, x_d, w_ds, pw_d, bias_d, y_d, repeat=1,
          phases=("conv", "attn", "proj")):
    from contextlib import ExitStack
    ctx = ExitStack()
    C = M * 128
    const = ctx.enter_context(tc.tile_pool(name="const", bufs=1))
    xpad_p = ctx.enter_context(tc.tile_pool(name="xpad", bufs=2))
    w_p = ctx.enter_context(tc.tile_pool(name="wconv", bufs=2))
    v_p = ctx.enter_context(tc.tile_pool(name="vwino", bufs=4))
    qkv_p = ctx.enter_context(tc.tile_pool(name="qkv", bufs=4))
    exp_p = ctx.enter_context(tc.tile_pool(name="exp", bufs=3))
    nt_p = ctx.enter_context(tc.tile_pool(name="normT", bufs=1))
    rc_p = ctx.enter_context(tc.tile_pool(name="recip", bufs=1))
    as_p = ctx.enter_context(tc.tile_pool(name="attns", bufs=1))
    acm_p = ctx.enter_context(tc.tile_pool(name="attncm", bufs=min(M, 4)))
    y_p = ctx.enter_context(tc.tile_pool(name="yout", bufs=2))
    st_p = ctx.enter_context(tc.tile_pool(name="stage", bufs=4))
    cp_ps = ctx.enter_context(tc.tile_pool(name="cpps", bufs=4, space="PSUM"))
    at_ps = ctx.enter_context(tc.tile_pool(name="atps", bufs=2, space="PSUM"))
    nm_ps = ctx.enter_context(tc.tile_pool(name="nmps", bufs=1, space="PSUM"))
    o2_ps = ctx.enter_context(tc.tile_pool(name="o2ps", bufs=1, space="PSUM"))

    # constants
    ones32 = const.tile([128, 32], BF16, tag="ones32")
    nc.gpsimd.memset(ones32[:, :], 1.0)
    # per-partition bias columns: col (ti*M + m) = bias[ti, m*128:(m+1)*128]
    bias_sb = const.tile([128, 4 * M], F32, tag="bias")
    pw_sb = const.tile([128, M * C], BF16, tag="pw")
    pw_loaded = [False]

    def load_xpad(b_, k4_):
        """memset borders + row-halved x DMA for one ci-chunk."""
        xp = xpad_p.tile([128, PW * PW], F16, tag="xpad")
        z = xp[:, :].rearrange("p (r c) -> p r c", c=PW)
        nc.gpsimd.memset(z[:, 0, :], 0.0)
        nc.gpsimd.memset(z[:, PW - 1, :], 0.0)
        nc.gpsimd.memset(z[:, :, 0], 0.0)
        nc.gpsimd.memset(z[:, :, PW - 1], 0.0)
        nc.sync.dma_start(out=z[:, 1:33, 1:H + 1],
                          in_=x_d[b_, k4_ * 128:(k4_ + 1) * 128, 0:32, :])
        nc.sync.dma_start(out=z[:, 33:H + 1, 1:H + 1],
                          in_=x_d[b_, k4_ * 128:(k4_ + 1) * 128, 32:H, :])
        return xp

    xpre = {}
    xpre_b = [None]

    bseq = [b for _ in range(repeat) for b in range(B)]
    for bi, b in enumerate(bseq):
        # ---- load x image b (fp16): zero-pad borders, then 1D Winograd
        # B1^T input transform along W (4 DVE ops per ci-chunk), shared by
        # the q/k/v GEMMs. V[j, r', tw] in fp16.
        SUB = mybir.AluOpType.subtract
        ADD = mybir.AluOpType.add
        vts = []
        for k4 in range(M):
            xp = xpre.pop(k4, None) if b == xpre_b[0] else None
            if xp is None:
                xp = load_xpad(b, k4)
            z = xp[:, :].rearrange("p (r c) -> p r c", c=PW)
            vt = v_p.tile([128, 4 * PW * 32], F16, tag="vtile")
            vz = vt[:, :].rearrange("p (j r c) -> p j r c", j=4, r=PW)
            z2 = xp[:, :].rearrange("p (r c2 par) -> p r c2 par", par=2, c2=33)
            rsplit = ((0, 19), (19, 33), (33, PW)) if k4 == 0 else \
                ((0, 33), (33, PW))
            for r0, r1 in rsplit:
                dA0 = z2[:, r0:r1, 0:32, 0]       # cp = 2tw
                dA1 = z2[:, r0:r1, 0:32, 1]       # cp = 2tw+1
                dB0 = z2[:, r0:r1, 1:33, 0]       # cp = 2tw+2
                dB1 = z2[:, r0:r1, 1:33, 1]       # cp = 2tw+3
                nc.vector.tensor_tensor(vz[:, 0, r0:r1], dA0, dB0, SUB)
                nc.vector.tensor_tensor(vz[:, 1, r0:r1], dA1, dB0, ADD)
                nc.vector.tensor_tensor(vz[:, 2, r0:r1], dB0, dA1, SUB)
                nc.vector.tensor_tensor(vz[:, 3, r0:r1], dA1, dB1, SUB)
            vts.append(vz)
            if k4 == 0:
                w_pre = w_p.tile([128, 12 * M * 128], F16, tag="wconv")
                for j in (1, 2, 0, 3):   # vgemm consumption order
                    nc.sync.dma_start(
                        out=w_pre[:, 3 * j * M * 128:3 * (j + 1) * M * 128],
                        in_=w_ds[0][0][:, 3 * j * M:3 * (j + 1) * M, :]
                        .rearrange("p a b -> p (a b)"))
            if k4 == 1 and not pw_loaded[0]:
                pw_loaded[0] = True
                nc.sync.dma_start(
                    out=bias_sb[:, :],
                    in_=bias_d[:, :].rearrange("a (m p) -> p (a m)", p=128))
                for kk in range(M):
                    nc.sync.dma_start(out=pw_sb[:, kk * C:(kk + 1) * C],
                                      in_=pw_d[kk, :, :])

        acm = []  # attnout channel-major chunks for proj
        for m in range(M):
            outs = {}

            def vgemm(wt, j, rsl, csl, cpr):
                """Accumulate M_j = sum_{dy,ci} U1[j,dy] @ V[j] into a psum.
                rsl/csl slice V rows/tile-cols; cpr=True puts tw outer
                (col-major free order for the v staging)."""
                ps = cp_ps.tile([128, 512], F32, tag="cpps")
                for i2, (k4, dy) in enumerate(
                        (k, d) for k in range(M) for d in range(3)):
                    rhs = vts[k4][:, j, dy + rsl:dy + rsl + (64 if cpr else 16),
                                  csl:csl + (8 if cpr else 32)]
                    if cpr:
                        rhs = rhs.transpose([0, 2, 1])
                    nc.tensor.matmul(
                        ps[:, :],
                        wt[:, ((j * 3 + dy) * M + k4) * 128:
                           ((j * 3 + dy) * M + k4 + 1) * 128],
                        rhs, start=(i2 == 0), stop=(i2 == 3 * M - 1))
                return ps

            def ycombine(wt, tb, o_t, qt, cpr):
                """Winograd output recombine for one psum group -> 2 STs.
                y0 = M0+M1+M2+b ; y1 = M1-M2-M3+b (fp32 PSUM reads)."""
                rsl = 0 if cpr else qt * 16
                csl = qt * 8 if cpr else 0
                shp = (lambda t: t.rearrange("p (tw r) -> p tw r", tw=8)) \
                    if cpr else \
                    (lambda t: t.rearrange("p (r tw) -> p r tw", tw=32))
                m1 = vgemm(wt, 1, rsl, csl, cpr)
                ev1 = st_p.tile([128, 512], BF16, tag="stage")
                nc.scalar.activation(ev1[:, :], m1[:, :], AF.Identity, bias=tb)
                m2 = vgemm(wt, 2, rsl, csl, cpr)
                tws = st_p.tile([128, 512], BF16, tag="stage")
                twd = st_p.tile([128, 512], BF16, tag="stage")
                nc.vector.tensor_tensor(shp(tws[:, :]), shp(ev1[:, :]),
                                        shp(m2[:, :]), ADD)
                nc.vector.tensor_tensor(shp(twd[:, :]), shp(ev1[:, :]),
                                        shp(m2[:, :]), SUB)
                m0 = vgemm(wt, 0, rsl, csl, cpr)
                m3 = vgemm(wt, 3, rsl, csl, cpr)
                stg = st_p.tile([128, 1024], BF16, tag="stage2")
                if cpr:   # col-major: free = (c 16, r 64), c = 2tw+q
                    sz = stg[:, :].rearrange("p (tw q r) -> p tw q r",
                                             tw=8, q=2)
                    y0, y1 = sz[:, :, 0, :], sz[:, :, 1, :]
                else:     # row-major: free = (r 16, c 64), c = 2c2+q
                    sz = stg[:, :].rearrange("p (r c2 q) -> p r c2 q",
                                             q=2, c2=32)
                    y0, y1 = sz[:, :, :, 0], sz[:, :, :, 1]
                nc.vector.tensor_tensor(y0, shp(tws[:, :]), shp(m0[:, :]), ADD)
                nc.vector.tensor_tensor(y1, shp(twd[:, :]), shp(m3[:, :]), SUB)
                nc.vector.transpose(o_t[:, (2 * qt) * 512:(2 * qt + 1) * 512],
                                    stg[:, 0:512])
                nc.vector.transpose(o_t[:, (2 * qt + 1) * 512:
                                        (2 * qt + 2) * 512],
                                    stg[:, 512:1024])

            for ti, tname in enumerate(("q", "k", "v")):
                if m == 0 and ti == 0:
                    wt = w_pre
                else:
                    wt = w_p.tile([128, 12 * M * 128], F16, tag="wconv")
                    nc.sync.dma_start(
                        out=wt[:, :],
                        in_=w_ds[ti][m].rearrange("p a b -> p (a b)"))
                o_t = qkv_p.tile([128, NPOS], BF16, tag="qkv")
                tb = bias_sb[:, ti * M + m: ti * M + m + 1]
                for qt in range(4):
                    ycombine(wt, tb, o_t, qt, cpr=(tname == "v"))
                outs[tname] = o_t

            if "attn" not in phases:
                nc.gpsimd.dma_start(
                    out=y_d[b, m * 128:(m + 1) * 128].rearrange("p a b -> p (a b)"),
                    in_=outs["q"][:, :])
                continue

            # ---- attention for the 128 channels of chunk m
            o_q, o_k, o_v = outs["q"], outs["k"], outs["v"]
            a_s = as_p.tile([128, NPOS], BF16, tag="attns")
            kks = [o_k[cb * 32:(cb + 1) * 32, :].rearrange(
                "p (kid half c) -> p kid half c", half=2, c=32) for cb in range(4)]
            qqs = [o_q[cb * 32:(cb + 1) * 32, :].rearrange(
                "p (i half c) -> p i half c", half=2, c=32) for cb in range(4)]
            vvs = [o_v[cb * 32:(cb + 1) * 32, :].rearrange(
                "p (w half c) -> p w half c", half=2, c=32) for cb in range(4)]
            for qd in range(8):
                atp = at_ps.tile([128, 512], F32, tag="atps")
                # logits^T:  atp[cb*32+kappa, sl*128+kb*64+i] = sum_j k*q
                # cb innermost so consecutive MMs hit different PE quadrants
                for i1, (sl, kb, jb) in enumerate(
                        (s, k, j) for s in range(4) for k in range(2) for j in range(2)):
                    c = qd * 4 + sl
                    for cb in range(4):
                        nc.tensor.matmul(
                            atp[cb * 32:(cb + 1) * 32,
                                sl * 128 + kb * 64: sl * 128 + (kb + 1) * 64],
                            kks[cb][:, kb * 32:(kb + 1) * 32, jb, c],
                            qqs[cb][:, :, jb, c],
                            start=(i1 == 0), stop=(i1 == 15),
                            skip_group_check=True,
                            tile_position=(cb * 32, cb * 32))
                # exp (fp32 -> bf16), no max subtraction
                ex = exp_p.tile([128, 512], BF16, tag="exp")
                nc.scalar.activation(ex[:, :], atp[:, :], AF.Exp)
                # row sums (over kidx) via ones-matmul, replicated on 32 parts
                nmp = nm_ps.tile([128, 256], F32, tag="nmps")
                for kb in range(2):
                    for cb in range(4):
                        ee = ex[cb * 32:(cb + 1) * 32, :].rearrange(
                            "p (sl half i) -> p sl half i", half=2, i=64)
                        nc.tensor.matmul(
                            nmp[cb * 32:(cb + 1) * 32, :],
                            ones32[cb * 32:(cb + 1) * 32, :],
                            ee[:, :, kb, :],
                            start=(kb == 0), stop=(kb == 1),
                            skip_group_check=True,
                            tile_position=(cb * 32, cb * 32))
                nt = nt_p.tile([128, 256], F32, tag="normT")
                nc.vector.transpose(nt[:, :], nmp[:, :])
                rc = rc_p.tile([128, 8], F32, tag="recip")
                nc.vector.reciprocal(
                    rc[:, :], nt[:, :].rearrange("p (t c) -> p t c", c=32)[:, :, 0])
                # out2 = attn_exp^T' @ v   (unnormalized), K=32 chunks
                o2p = o2_ps.tile([128, 512], F32, tag="o2ps")
                for i2, (sl, ib, kb) in enumerate(
                        (s, i, k) for s in range(4) for i in range(2) for k in range(2)):
                    c = qd * 4 + sl
                    for cb in range(4):
                        nc.tensor.matmul(
                            o2p[cb * 32:(cb + 1) * 32,
                                sl * 128 + ib * 64: sl * 128 + (ib + 1) * 64],
                            ex[cb * 32:(cb + 1) * 32,
                               sl * 128 + kb * 64 + ib * 32:
                               sl * 128 + kb * 64 + ib * 32 + 32],
                            vvs[cb][:, :, kb, c],
                            start=(i2 == 0), stop=(i2 == 15),
                            skip_group_check=True,
                            tile_position=(cb * 32, cb * 32))
                # normalize + write into attnout_s (v-style layout), bf16
                in0 = o2p[:, :].rearrange("p (sl ib w) -> p sl ib w", ib=2, w=64)
                in1 = rc[:, :].rearrange("p (sl ib) -> p sl ib", ib=2)
                in1 = in1.unsqueeze(3).broadcast_to((128, 4, 2, 64))
                outap = a_s[:, :].rearrange("p (t c) -> p t c", c=32)
                outap = outap[:, :, qd * 4:qd * 4 + 4].rearrange(
                    "p (w ib) sl -> p w ib sl", ib=2).transpose([0, 3, 2, 1])
                nc.vector.tensor_tensor(outap, in0, in1, mybir.AluOpType.mult)
            # back-transpose to channel-major (column-major positions)
            a_cm = acm_p.tile([128, NPOS], BF16, tag="attncm")
            nc.vector.transpose(a_cm[:, :], a_s[:, :])
            acm.append(a_cm)

        if "attn" not in phases:
            continue
        if "proj" not in phases:
            for m in range(M):
                nc.gpsimd.dma_start(
                    out=y_d[b, m * 128:(m + 1) * 128].rearrange("p a b -> p (a b)"),
                    in_=acm[m][:, :])
            del acm
            continue

        if bi + 1 < len(bseq):
            xpre_b[0] = bseq[bi + 1]
            for k4 in (0, 1):
                xpre[k4] = load_xpad(bseq[bi + 1], k4)

        # ---- proj (1x1 conv with permuted weights) + bias, row-major out.
        # y-writes batched in pairs of psum groups (one 512 KB DMA per 16
        # output rows) to halve the per-DMA overhead on the DMA engines.
        for mo in range(M):
            for n2 in range(4):
                yt = y_p.tile([128, NPOS // 4], F32, tag="yout")
                for half in range(2):
                    n = n2 * 2 + half
                    psum = cp_ps.tile([128, 512], F32, tag="cpps")
                    for k4 in range(M):
                        rhs = acm[k4][:, :].rearrange("p (w i) -> p w i", i=64)
                        rhs = rhs[:, :, n * 8:(n + 1) * 8].transpose([0, 2, 1])
                        nc.tensor.matmul(
                            psum[:, :],
                            pw_sb[:, k4 * C + mo * 128: k4 * C + (mo + 1) * 128],
                            rhs, start=(k4 == 0), stop=(k4 == M - 1))
                    nc.scalar.activation(
                        yt[:, half * 512:(half + 1) * 512], psum[:, :],
                        AF.Identity,
                        bias=bias_sb[:, 3 * M + mo: 3 * M + mo + 1])
                if b == B - 1 and mo == M - 1 and n2 == 3:
                    nc.sync.dma_start(
                        out=y_d[b, mo * 128:(mo + 1) * 128, 48:56, :],
                        in_=yt[:, 0:512])
                    nc.sync.dma_start(
                        out=y_d[b, mo * 128:(mo + 1) * 128, 56:64, :],
                        in_=yt[:, 512:1024])
                else:
                    nc.sync.dma_start(
                        out=y_d[b, mo * 128:(mo + 1) * 128,
                                n2 * 16:(n2 + 1) * 16, :],
                        in_=yt[:, :])
        del acm
    ctx.close()


def prep_weights(q_w, q_b, kv_w, kv_b, proj_w, proj_b, C=512):
    """Host-side weight re-layouts (numpy, bf16)."""
    M = C // 128
    nh = 16
    cpg = C // nh

    def conv_w(w):
        # w[co, ci, dy, dx] -> [m, p(ci%128), t(=dy*3+dx), k4, co] flat
        w4 = w.reshape(M, 128, M, 128, 3, 3)          # [m, co, k4, p, dy, dx]
        out = np.transpose(w4, (0, 3, 4, 5, 2, 1))    # [m, p, dy, dx, k4, co]
        out = out.reshape(M, 128, 9 * M, 128)
        return np.ascontiguousarray(out).astype(BF)

    def conv_w_wino(w):
        # 1D Winograd F(2,3) along W: U1[j,dy,o,c] = sum_dx G1[j,dx] w[o,c,dy,dx]
        # layout [m, p(ci%128), t'(=j*3+dy), k4, co] flat
        G1 = np.array([[1, 0, 0], [.5, .5, .5], [.5, -.5, .5], [0, 0, 1]],
                      np.float32)
        U1 = np.einsum('jx,ocdx->ocjd', G1, w.astype(np.float32))
        w4 = U1.reshape(M, 128, M, 128, 4, 3)         # [m, co, k4, p, j, dy]
        out = np.transpose(w4, (0, 3, 4, 5, 2, 1))    # [m, p, j, dy, k4, co]
        out = out.reshape(M, 128, 12 * M, 128)
        return np.ascontiguousarray(out).astype(np.float16)

    wq = conv_w_wino(q_w)
    wk = conv_w_wino(kv_w[:C])
    wv = conv_w_wino(kv_w[C:])
    ch = np.arange(C)
    perm = (ch % cpg) * nh + ch // cpg                # proj input index per attn channel
    pwp = proj_w[:, :, 0, 0][:, perm]                 # [co, ch]
    pw = np.ascontiguousarray(pwp.T.reshape(M, 128, C)).astype(BF)
    biases = np.stack([q_b, kv_b[:C], kv_b[C:], proj_b]).astype(np.float32)
    return wq, wk, wv, pw, biases


_CACHE = {}


def _get_nc():
    if "nc" not in _CACHE:
        _CACHE["nc"] = build_nc(B=2, C=512, n_cores=8)
    return _CACHE["nc"]


def make_in_maps(x, q_w, q_b, kv_w, kv_b, proj_w, proj_b, n_cores=8):
    wq, wk, wv, pw, biases = prep_weights(
        np.asarray(q_w), np.asarray(q_b), np.asarray(kv_w), np.asarray(kv_b),
        np.asarray(proj_w), np.asarray(proj_b))
    x = np.asarray(x, dtype=np.float32).astype(np.float16)
    bpc = x.shape[0] // n_cores
    return [
        {"x": np.ascontiguousarray(x[i * bpc:(i + 1) * bpc]),
         "wq": wq, "wk": wk, "wv": wv, "pw": pw, "biases": biases}
        for i in range(n_cores)
    ]


def kernel(x, q_w, q_b, kv_w, kv_b, proj_w, proj_b):
    nc = _get_nc()
    in_maps = make_in_maps(x, q_w, q_b, kv_w, kv_b, proj_w, proj_b)
    res = run_bass_kernel_spmd(nc, in_maps, core_ids=list(range(8)))
    out = np.concatenate([res.results[i]["y"] for i in range(8)], axis=0)
    return out.astype(np.float32)



# revision 39
# speedup vs baseline: 1.0106x; 1.0001x over previous
"""Trainium2 Bass kernel for nn_Attention (conv-qkv spatial attention block).

Contract: kernel(**inputs) takes FULL unsharded inputs (B=16, C=512, H=W=64),
shards batch across 8 NeuronCores (2 images per core), runs one SPMD Bass
program, and returns the FULL output (fp32).

Math per image (reference):
  q  = conv3x3(x, q_w) + q_b                      # (C, H, W)
  kv = conv3x3(x, kv_w) + kv_b ; k, v = split(kv)
  per channel ch: attn = softmax(q_ch @ k_ch^T) ; o_ch = attn @ v_ch
  y  = conv1x1(perm(o), proj_w) + proj_b          # head/channel permutation
       (the permutation is folded into proj_w on the host)

Device implementation notes:
  - All three 3x3 convs use 1D Winograd F(2,3) along W (1.5x fewer MACs):
    weights are G-transformed on the host (U1[j,dy] = sum_dx G1[j,dx] w),
    the input B1^T transform runs once per image on DVE (4 tensor_tensor
    ops per ci-chunk over a zero-padded copy of x) and is shared by q/k/v;
    the GEMM contracts (ci, dy) per j-position in fp16 with fp32 PSUM;
    output pairs recombine from PSUM in fp32 (y0=M0+M1+M2+b, y1=M1-M2-M3+b)
    with the bias folded into an ACT evacuation of M1.
  - Conv internals (x, weights, V) are fp16 — same PE speed as bf16 but 8x
    finer mantissa, which suppresses the Winograd noise amplification that
    would otherwise break the peaked-softmax logits. Attention operands and
    all attention matmuls stay bf16 (exp values overflow fp16 range).
  - Per-channel attention operands are produced by DVE stream-transpose
    (32x32 blocks), giving a tiled layout where the spatial index lives on
    partitions mod 32 and attention runs as K=32 matmuls packed 4-wide on
    the PE array via tile_position quadrants.
  - softmax: exp in fp32 without max subtraction (logits bounded ~|75| < 88),
    row sums via a ones-matmul, one reciprocal + broadcast multiply.
  - Keep the bias/pw DMAs on the sync (HWDGE) queue: the SWDGE (gpsimd)
    queue corrupts the rearranged bias load on the execution backend (NaN).
"""

import numpy as np
import ml_dtypes

import concourse.bass as bass
import concourse.bacc as bacc
import concourse.mybir as mybir
import concourse.tile as tile
from concourse.bass_utils import run_bass_kernel_spmd

F32 = mybir.dt.float32
BF16 = mybir.dt.bfloat16
F16 = mybir.dt.float16
AF = mybir.ActivationFunctionType
BF = ml_dtypes.bfloat16

H = 64          # spatial height (attention over rows, contracting cols)
PW = 66         # padded row width
NPOS = H * H    # 4096 positions per image


def build_nc(B=2, C=512, n_cores=8, repeat=1, phases=("conv", "attn", "proj")):
    """Build the per-core Bass program. B = images per core.

    repeat > 1 emits the whole body multiple times (timing builds only).
    phases: drop "attn"/"proj" for timing-breakdown builds.
    """
    M = C // 128            # channel chunks (co chunks and ci chunks)
    nc = bacc.Bacc("TRN2", target_bir_lowering=False, debug=False,
                   num_devices=n_cores)

    x_d = nc.dram_tensor("x", [B, C, H, H], F16, kind="ExternalInput")
    wq_d = nc.dram_tensor("wq", [M, 128, 12 * M, 128], F16, kind="ExternalInput")
    wk_d = nc.dram_tensor("wk", [M, 128, 12 * M, 128], F16, kind="ExternalInput")
    # v conv uses 1D Winograd F(2,3) along W: 12 = 4 j-positions x 3 dy taps
    wv_d = nc.dram_tensor("wv", [M, 128, 12 * M, 128], F16, kind="ExternalInput")
    pw_d = nc.dram_tensor("pw", [M, 128, C], BF16, kind="ExternalInput")
    bias_d = nc.dram_tensor("biases", [4, C], F32, kind="ExternalInput")
    y_d = nc.dram_tensor("y", [B, C, H, H], F32, kind="ExternalOutput")

    with tile.TileContext(nc) as tc:
        _body(tc, nc, B, M, x_d, (wq_d, wk_d, wv_d), pw_d, bias_d, y_d,
              repeat=repeat, phases=phases)
    nc.compile()
    return nc


def _body(tc, nc, B, M, x_d, w_ds, pw_d, bias_d, y_d, repeat=1,
          phases=("conv", "attn", "proj")):
    from contextlib import ExitStack
    ctx = ExitStack()
    C = M * 128
    const = ctx.enter_context(tc.tile_pool(name="const", bufs=1))
    xpad_p = ctx.enter_context(tc.tile_pool(name="xpad", bufs=2))
    w_p = ctx.enter_context(tc.tile_pool(name="wconv", bufs=2))
    v_p = ctx.enter_context(tc.tile_pool(name="vwino", bufs=4))
    qkv_p = ctx.enter_context(tc.tile_pool(name="qkv", bufs=4))
    exp_p = ctx.enter_context(tc.tile_pool(name="exp", bufs=3))
    nt_p = ctx.enter_context(tc.tile_pool(name="normT", bufs=1))
    rc_p = ctx.enter_context(tc.tile_pool(name="recip", bufs=1))
    as_p = ctx.enter_context(tc.tile_pool(name="attns", bufs=1))
    acm_p = ctx.enter_context(tc.tile_pool(name="attncm", bufs=min(M, 4)))
    y_p = ctx.enter_context(tc.tile_pool(name="yout", bufs=2))
    st_p = ctx.enter_context(tc.tile_pool(name="stage", bufs=4))
    cp_ps = ctx.enter_context(tc.tile_pool(name="cpps", bufs=4, space="PSUM"))
    at_ps = ctx.enter_context(tc.tile_pool(name="atps", bufs=2, space="PSUM"))
    nm_ps = ctx.enter_context(tc.tile_pool(name="nmps", bufs=1, space="PSUM"))
    o2_ps = ctx.enter_context(tc.tile_pool(name="o2ps", bufs=1, space="PSUM"))

    # constants
    ones32 = const.tile([128, 32], BF16, tag="ones32")
    nc.gpsimd.memset(ones32[:, :], 1.0)
    # per-partition bias columns: col (ti*M + m) = bias[ti, m*128:(m+1)*128]
    bias_sb = const.tile([128, 4 * M], F32, tag="bias")
    pw_sb = const.tile([128, M * C], BF16, tag="pw")
    pw_loaded = [False]

    def load_xpad(b_, k4_):
        """memset borders + row-halved x DMA for one ci-chunk."""
        xp = xpad_p.tile([128, PW * PW], F16, tag="xpad")
        z = xp[:, :].rearrange("p (r c) -> p r c", c=PW)
        nc.gpsimd.memset(z[:, 0, :], 0.0)
        nc.gpsimd.memset(z[:, PW - 1, :], 0.0)
        nc.gpsimd.memset(z[:, :, 0], 0.0)
        nc.gpsimd.memset(z[:, :, PW - 1], 0.0)
        nc.sync.dma_start(out=z[:, 1:33, 1:H + 1],
                          in_=x_d[b_, k4_ * 128:(k4_ + 1) * 128, 0:32, :])
        nc.sync.dma_start(out=z[:, 33:H + 1, 1:H + 1],
                          in_=x_d[b_, k4_ * 128:(k4_ + 1) * 128, 32:H, :])
        return xp

    xpre = {}
    xpre_b = [None]

    bseq = [b for _ in range(repeat) for b in range(B)]
    for bi, b in enumerate(bseq):
        # ---- load x image b (fp16): zero-pad borders, then 1D Winograd
        # B1^T input transform along W (4 DVE ops per ci-chunk), shared by
        # the q/k/v GEMMs. V[j, r', tw] in fp16.
        SUB = mybir.AluOpType.subtract
        ADD = mybir.AluOpType.add
        vts = []
        for k4 in range(M):
            xp = xpre.pop(k4, None) if b == xpre_b[0] else None
            if xp is None:
                xp = load_xpad(b, k4)
            z = xp[:, :].rearrange("p (r c) -> p r c", c=PW)
            vt = v_p.tile([128, 4 * PW * 32], F16, tag="vtile")
            vz = vt[:, :].rearrange("p (j r c) -> p j r c", j=4, r=PW)
            z2 = xp[:, :].rearrange("p (r c2 par) -> p r c2 par", par=2, c2=33)
            rsplit = ((0, 19), (19, 33), (33, PW)) if k4 == 0 else \
                ((0, 33), (33, PW))
            for r0, r1 in rsplit:
                dA0 = z2[:, r0:r1, 0:32, 0]       # cp = 2tw
                dA1 = z2[:, r0:r1, 0:32, 1]       # cp = 2tw+1
                dB0 = z2[:, r0:r1, 1:33, 0]       # cp = 2tw+2
                dB1 = z2[:, r0:r1, 1:33, 1]       # cp = 2tw+3
                # emitted in vgemm consumption order: j = 1, 2, 0, 3
                nc.vector.tensor_tensor(vz[:, 1, r0:r1], dA1, dB0, ADD)
                nc.vector.tensor_tensor(vz[:, 2, r0:r1], dB0, dA1, SUB)
                nc.vector.tensor_tensor(vz[:, 0, r0:r1], dA0, dB0, SUB)
                nc.vector.tensor_tensor(vz[:, 3, r0:r1], dA1, dB1, SUB)
            vts.append(vz)
            if k4 == 0:
                w_pre = w_p.tile([128, 12 * M * 128], F16, tag="wconv")
                for j in (1, 2, 0, 3):   # vgemm consumption order
                    nc.sync.dma_start(
                        out=w_pre[:, 3 * j * M * 128:3 * (j + 1) * M * 128],
                        in_=w_ds[0][0][:, 3 * j * M:3 * (j + 1) * M, :]
                        .rearrange("p a b -> p (a b)"))
            if k4 == 1 and not pw_loaded[0]:
                pw_loaded[0] = True
                nc.sync.dma_start(
                    out=bias_sb[:, :],
                    in_=bias_d[:, :].rearrange("a (m p) -> p (a m)", p=128))
                for kk in range(M):
                    nc.sync.dma_start(out=pw_sb[:, kk * C:(kk + 1) * C],
                                      in_=pw_d[kk, :, :])

        acm = []  # attnout channel-major chunks for proj
        for m in range(M):
            outs = {}

            def vgemm(wt, j, rsl, csl, cpr):
                """Accumulate M_j = sum_{dy,ci} U1[j,dy] @ V[j] into a psum.
                rsl/csl slice V rows/tile-cols; cpr=True puts tw outer
                (col-major free order for the v staging)."""
                ps = cp_ps.tile([128, 512], F32, tag="cpps")
                for i2, (k4, dy) in enumerate(
                        (k, d) for k in range(M) for d in range(3)):
                    rhs = vts[k4][:, j, dy + rsl:dy + rsl + (64 if cpr else 16),
                                  csl:csl + (8 if cpr else 32)]
                    if cpr:
                        rhs = rhs.transpose([0, 2, 1])
                    nc.tensor.matmul(
                        ps[:, :],
                        wt[:, ((j * 3 + dy) * M + k4) * 128:
                           ((j * 3 + dy) * M + k4 + 1) * 128],
                        rhs, start=(i2 == 0), stop=(i2 == 3 * M - 1))
                return ps

            def ycombine(wt, tb, o_t, qt, cpr):
                """Winograd output recombine for one psum group -> 2 STs.
                y0 = M0+M1+M2+b ; y1 = M1-M2-M3+b (fp32 PSUM reads)."""
                rsl = 0 if cpr else qt * 16
                csl = qt * 8 if cpr else 0
                shp = (lambda t: t.rearrange("p (tw r) -> p tw r", tw=8)) \
                    if cpr else \
                    (lambda t: t.rearrange("p (r tw) -> p r tw", tw=32))
                m1 = vgemm(wt, 1, rsl, csl, cpr)
                ev1 = st_p.tile([128, 512], BF16, tag="stage")
                nc.scalar.activation(ev1[:, :], m1[:, :], AF.Identity, bias=tb)
                m2 = vgemm(wt, 2, rsl, csl, cpr)
                tws = st_p.tile([128, 512], BF16, tag="stage")
                twd = st_p.tile([128, 512], BF16, tag="stage")
                nc.vector.tensor_tensor(shp(tws[:, :]), shp(ev1[:, :]),
                                        shp(m2[:, :]), ADD)
                nc.vector.tensor_tensor(shp(twd[:, :]), shp(ev1[:, :]),
                                        shp(m2[:, :]), SUB)
                m0 = vgemm(wt, 0, rsl, csl, cpr)
                m3 = vgemm(wt, 3, rsl, csl, cpr)
                stg = st_p.tile([128, 1024], BF16, tag="stage2")
                if cpr:   # col-major: free = (c 16, r 64), c = 2tw+q
                    sz = stg[:, :].rearrange("p (tw q r) -> p tw q r",
                                             tw=8, q=2)
                    y0, y1 = sz[:, :, 0, :], sz[:, :, 1, :]
                else:     # row-major: free = (r 16, c 64), c = 2c2+q
                    sz = stg[:, :].rearrange("p (r c2 q) -> p r c2 q",
                                             q=2, c2=32)
                    y0, y1 = sz[:, :, :, 0], sz[:, :, :, 1]
                nc.vector.tensor_tensor(y0, shp(tws[:, :]), shp(m0[:, :]), ADD)
                nc.vector.tensor_tensor(y1, shp(twd[:, :]), shp(m3[:, :]), SUB)
                nc.vector.transpose(o_t[:, (2 * qt) * 512:(2 * qt + 1) * 512],
                                    stg[:, 0:512])
                nc.vector.transpose(o_t[:, (2 * qt + 1) * 512:
                                        (2 * qt + 2) * 512],
                                    stg[:, 512:1024])

            for ti, tname in enumerate(("q", "k", "v")):
                if m == 0 and ti == 0:
                    wt = w_pre
                else:
                    wt = w_p.tile([128, 12 * M * 128], F16, tag="wconv")
                    nc.sync.dma_start(
                        out=wt[:, :],
                        in_=w_ds[ti][m].rearrange("p a b -> p (a b)"))
                o_t = qkv_p.tile([128, NPOS], BF16, tag="qkv")
                tb = bias_sb[:, ti * M + m: ti * M + m + 1]
                for qt in range(4):
                    ycombine(wt, tb, o_t, qt, cpr=(tname == "v"))
                outs[tname] = o_t

            if "attn" not in phases:
                nc.gpsimd.dma_start(
                    out=y_d[b, m * 128:(m + 1) * 128].rearrange("p a b -> p (a b)"),
                    in_=outs["q"][:, :])
                continue

            # ---- attention for the 128 channels of chunk m
            o_q, o_k, o_v = outs["q"], outs["k"], outs["v"]
            a_s = as_p.tile([128, NPOS], BF16, tag="attns")
            kks = [o_k[cb * 32:(cb + 1) * 32, :].rearrange(
                "p (kid half c) -> p kid half c", half=2, c=32) for cb in range(4)]
            qqs = [o_q[cb * 32:(cb + 1) * 32, :].rearrange(
                "p (i half c) -> p i half c", half=2, c=32) for cb in range(4)]
            vvs = [o_v[cb * 32:(cb + 1) * 32, :].rearrange(
                "p (w half c) -> p w half c", half=2, c=32) for cb in range(4)]
            for qd in range(8):
                atp = at_ps.tile([128, 512], F32, tag="atps")
                # logits^T:  atp[cb*32+kappa, sl*128+kb*64+i] = sum_j k*q
                # cb innermost so consecutive MMs hit different PE quadrants
                for i1, (sl, kb, jb) in enumerate(
                        (s, k, j) for s in range(4) for k in range(2) for j in range(2)):
                    c = qd * 4 + sl
                    for cb in range(4):
                        nc.tensor.matmul(
                            atp[cb * 32:(cb + 1) * 32,
                                sl * 128 + kb * 64: sl * 128 + (kb + 1) * 64],
                            kks[cb][:, kb * 32:(kb + 1) * 32, jb, c],
                            qqs[cb][:, :, jb, c],
                            start=(i1 == 0), stop=(i1 == 15),
                            skip_group_check=True,
                            tile_position=(cb * 32, cb * 32))
                # exp (fp32 -> bf16), no max subtraction
                ex = exp_p.tile([128, 512], BF16, tag="exp")
                nc.scalar.activation(ex[:, :], atp[:, :], AF.Exp)
                # row sums (over kidx) via ones-matmul, replicated on 32 parts
                nmp = nm_ps.tile([128, 256], F32, tag="nmps")
                for kb in range(2):
                    for cb in range(4):
                        ee = ex[cb * 32:(cb + 1) * 32, :].rearrange(
                            "p (sl half i) -> p sl half i", half=2, i=64)
                        nc.tensor.matmul(
                            nmp[cb * 32:(cb + 1) * 32, :],
                            ones32[cb * 32:(cb + 1) * 32, :],
                            ee[:, :, kb, :],
                            start=(kb == 0), stop=(kb == 1),
                            skip_group_check=True,
                            tile_position=(cb * 32, cb * 32))
                nt = nt_p.tile([128, 256], F32, tag="normT")
                nc.vector.transpose(nt[:, :], nmp[:, :])
                rc = rc_p.tile([128, 8], F32, tag="recip")
                nc.vector.reciprocal(
                    rc[:, :], nt[:, :].rearrange("p (t c) -> p t c", c=32)[:, :, 0])
                # out2 = attn_exp^T' @ v   (unnormalized), K=32 chunks
                o2p = o2_ps.tile([128, 512], F32, tag="o2ps")
                for i2, (sl, ib, kb) in enumerate(
                        (s, i, k) for s in range(4) for i in range(2) for k in range(2)):
                    c = qd * 4 + sl
                    for cb in range(4):
                        nc.tensor.matmul(
                            o2p[cb * 32:(cb + 1) * 32,
                                sl * 128 + ib * 64: sl * 128 + (ib + 1) * 64],
                            ex[cb * 32:(cb + 1) * 32,
                               sl * 128 + kb * 64 + ib * 32:
                               sl * 128 + kb * 64 + ib * 32 + 32],
                            vvs[cb][:, :, kb, c],
                            start=(i2 == 0), stop=(i2 == 15),
                            skip_group_check=True,
                            tile_position=(cb * 32, cb * 32))
                # normalize + write into attnout_s (v-style layout), bf16
                in0 = o2p[:, :].rearrange("p (sl ib w) -> p sl ib w", ib=2, w=64)
                in1 = rc[:, :].rearrange("p (sl ib) -> p sl ib", ib=2)
                in1 = in1.unsqueeze(3).broadcast_to((128, 4, 2, 64))
                outap = a_s[:, :].rearrange("p (t c) -> p t c", c=32)
                outap = outap[:, :, qd * 4:qd * 4 + 4].rearrange(
                    "p (w ib) sl -> p w ib sl", ib=2).transpose([0, 3, 2, 1])
                nc.vector.tensor_tensor(outap, in0, in1, mybir.AluOpType.mult)
            # back-transpose to channel-major (column-major positions)
            a_cm = acm_p.tile([128, NPOS], BF16, tag="attncm")
            nc.vector.transpose(a_cm[:, :], a_s[:, :])
            acm.append(a_cm)

        if "attn" not in phases:
            continue
        if "proj" not in phases:
            for m in range(M):
                nc.gpsimd.dma_start(
                    out=y_d[b, m * 128:(m + 1) * 128].rearrange("p a b -> p (a b)"),
                    in_=acm[m][:, :])
            del acm
            continue

        if bi + 1 < len(bseq):
            xpre_b[0] = bseq[bi + 1]
            for k4 in (0, 1):
                xpre[k4] = load_xpad(bseq[bi + 1], k4)

        # ---- proj (1x1 conv with permuted weights) + bias, row-major out.
        # y-writes batched in pairs of psum groups (one 512 KB DMA per 16
        # output rows) to halve the per-DMA overhead on the DMA engines.
        for mo in range(M):
            for n2 in range(4):
                yt = y_p.tile([128, NPOS // 4], F32, tag="yout")
                for half in range(2):
                    n = n2 * 2 + half
                    psum = cp_ps.tile([128, 512], F32, tag="cpps")
                    for k4 in range(M):
                        rhs = acm[k4][:, :].rearrange("p (w i) -> p w i", i=64)
                        rhs = rhs[:, :, n * 8:(n + 1) * 8].transpose([0, 2, 1])
                        nc.tensor.matmul(
                            psum[:, :],
                            pw_sb[:, k4 * C + mo * 128: k4 * C + (mo + 1) * 128],
                            rhs, start=(k4 == 0), stop=(k4 == M - 1))
                    nc.scalar.activation(
                        yt[:, half * 512:(half + 1) * 512], psum[:, :],
                        AF.Identity,
                        bias=bias_sb[:, 3 * M + mo: 3 * M + mo + 1])
                if b == B - 1 and mo == M - 1 and n2 == 3:
                    nc.sync.dma_start(
                        out=y_d[b, mo * 128:(mo + 1) * 128, 48:56, :],
                        in_=yt[:, 0:512])
                    nc.sync.dma_start(
                        out=y_d[b, mo * 128:(mo + 1) * 128, 56:64, :],
                        in_=yt[:, 512:1024])
                else:
                    nc.sync.dma_start(
                        out=y_d[b, mo * 128:(mo + 1) * 128,
                                n2 * 16:(n2 + 1) * 16, :],
                        in_=yt[:, :])
        del acm
    ctx.close()


def prep_weights(q_w, q_b, kv_w, kv_b, proj_w, proj_b, C=512):
    """Host-side weight re-layouts (numpy, bf16)."""
    M = C // 128
    nh = 16
    cpg = C // nh

    def conv_w(w):
        # w[co, ci, dy, dx] -> [m, p(ci%128), t(=dy*3+dx), k4, co] flat
        w4 = w.reshape(M, 128, M, 128, 3, 3)          # [m, co, k4, p, dy, dx]
        out = np.transpose(w4, (0, 3, 4, 5, 2, 1))    # [m, p, dy, dx, k4, co]
        out = out.reshape(M, 128, 9 * M, 128)
        return np.ascontiguousarray(out).astype(BF)

    def conv_w_wino(w):
        # 1D Winograd F(2,3) along W: U1[j,dy,o,c] = sum_dx G1[j,dx] w[o,c,dy,dx]
        # layout [m, p(ci%128), t'(=j*3+dy), k4, co] flat
        G1 = np.array([[1, 0, 0], [.5, .5, .5], [.5, -.5, .5], [0, 0, 1]],
                      np.float32)
        U1 = np.einsum('jx,ocdx->ocjd', G1, w.astype(np.float32))
        w4 = U1.reshape(M, 128, M, 128, 4, 3)         # [m, co, k4, p, j, dy]
        out = np.transpose(w4, (0, 3, 4, 5, 2, 1))    # [m, p, j, dy, k4, co]
        out = out.reshape(M, 128, 12 * M, 128)
        return np.ascontiguousarray(out).astype(np.float16)

    wq = conv_w_wino(q_w)
    wk = conv_w_wino(kv_w[:C])
    wv = conv_w_wino(kv_w[C:])
    ch = np.arange(C)
    perm = (ch % cpg) * nh + ch // cpg                # proj input index per attn channel
    pwp = proj_w[:, :, 0, 0][:, perm]                 # [co, ch]
    pw = np.ascontiguousarray(pwp.T.reshape(M, 128, C)).astype(BF)
    biases = np.stack([q_b, kv_b[:C], kv_b[C:], proj_b]).astype(np.float32)
    return wq, wk, wv, pw, biases


_CACHE = {}


def _get_nc():
    if "nc" not in _CACHE:
        _CACHE["nc"] = build_nc(B=2, C=512, n_cores=8)
    return _CACHE["nc"]


def make_in_maps(x, q_w, q_b, kv_w, kv_b, proj_w, proj_b, n_cores=8):
    wq, wk, wv, pw, biases = prep_weights(
        np.asarray(q_w), np.asarray(q_b), np.asarray(kv_w), np.asarray(kv_b),
        np.asarray(proj_w), np.asarray(proj_b))
    x = np.asarray(x, dtype=np.float32).astype(np.float16)
    bpc = x.shape[0] // n_cores
    return [
        {"x": np.ascontiguousarray(x[i * bpc:(i + 1) * bpc]),
         "wq": wq, "wk": wk, "wv": wv, "pw": pw, "biases": biases}
        for i in range(n_cores)
    ]


def kernel(x, q_w, q_b, kv_w, kv_b, proj_w, proj_b):
    nc = _get_nc()
    in_maps = make_in_maps(x, q_w, q_b, kv_w, kv_b, proj_w, proj_b)
    res = run_bass_kernel_spmd(nc, in_maps, core_ids=list(range(8)))
    out = np.concatenate([res.results[i]["y"] for i in range(8)], axis=0)
    return out.astype(np.float32)

